# revision 1
# baseline (speedup 1.0000x reference)
"""DIFF-Attention Trainium2 kernel (v3).

Problem: B=2, N=2048, DIM=768, H=12, HD=64, two qkv projections, two
softmax attention maps, diff = attn1 - lam*attn2, out = diff @ v1,
RMSNorm, proj.

Sharding: 8 cores; core c handles batch b = c//4 and query tokens
[512*(c%4), 512*(c%4)+512). Attention is permutation-invariant over
keys, so the host hands each core x^T with the token axis ROTATED so
that the core's 512 query tokens come first: q GEMMs read the first
512 columns of the same xT tile the k/v GEMMs read (no separate xq
DMA). Each core computes k1/k2/v1 for the whole (permuted) batch and
q/attention/norm/proj only for its 512 query tokens. No collectives.

Structure:
  - Each attention chain runs its 8 QK+exp groups first (8 E tiles
    buffered), then 4 query-tile AV passes with out[query, hd]
    (queries on psum partitions, 65-wide moving dim). Sequential
    per-region psum accumulation — interleaved start/stop groups
    within one psum bank silently corrupt results.
  - combine() reads the AV psums directly (no PE transpose of O).
  - v1 is computed per head-pair (prefetched like k/q) as a bf16
    token-major GEMM from an on-chip bf16 copy of x^T.
  - Work units (next-pair k/q/v GEMMs, prev-chunk Y transpose+stats)
    are popped one per attention group. Units only needed by LATER
    chains (q2/k2 fills, chunk finish) are carryable across the pair
    boundary so the last pairs keep PE fed.
  - Tail pipelines per query tile: Y-chunk-5 transpose -> rms -> proj
    -> epilogue -> half-row output DMAs.

Numerics: f32r (tf32-like) qkv/QK GEMMs, bf16 exp(S) / AV / v1 / proj,
fp32 PSUM accumulation, RMSNorm stats in fp32.
"""

import os
import numpy as np

B, N, DIM, H, HD = 2, 2048, 768, 12, 64
NQ = 512            # query tokens per core
LAMBDA_INIT = 0.1
EPS = 1e-6
NCORES = 8

_cache = {}


def _split_waits(nc, max_waits=1):
    """The walrus build in this environment rejects instructions carrying
    more than one explicit sync wait. Hoist excess waits onto NoOps
    inserted just before, on the same engine (same-engine program order
    makes this semantically equivalent)."""
    import concourse.mybir as mybir

    ctr = 0
    for f in nc.m.functions:
        for b in f.blocks:
            out = []
            changed = False
            for inst in b.instructions:
                si = inst.sync_info
                waits = list(si.on_wait) if si is not None and si.on_wait else []
                if len(waits) > max_waits:
                    changed = True
                    keep = waits[-max_waits:]
                    excess = waits[:-max_waits]
                    for i in range(0, len(excess), max_waits):
                        ctr += 1
                        nop = mybir.InstNoOp(
                            name=f"I-waitsplit-{ctr}", ins=[], outs=[]
                        )
                        nop.engine = inst.engine
                        nop.sync_info = mybir.SyncInfo(
                            on_wait=excess[i : i + max_waits], on_update=[]
                        )
                        out.append(nop)
                    inst.sync_info = mybir.SyncInfo(
                        on_wait=keep,
                        on_update=list(si.on_update) if si.on_update else [],
                    )
                out.append(inst)
            if changed:
                b.instructions = out


def _build():
    import concourse.bass as bass
    import concourse.mybir as mybir
    import concourse.tile as tile
    from concourse.masks import make_identity
    from collections import deque

    f32 = mybir.dt.float32
    f32r = mybir.dt.float32r
    bf16 = mybir.dt.bfloat16

    nc = bass.Bass(trn_type="TRN2")

    xT_d = nc.dram_tensor("xTp", [DIM, N], f32r, kind="ExternalInput")
    w1_d = nc.dram_tensor("w1", [DIM, 2 * DIM], f32r, kind="ExternalInput")
    w2_d = nc.dram_tensor("w2", [DIM, 2 * DIM], f32r, kind="ExternalInput")
    wv1_d = nc.dram_tensor("wv1b", [DIM, DIM], bf16, kind="ExternalInput")
    wp_d = nc.dram_tensor("wp", [DIM, DIM], bf16, kind="ExternalInput")
    bp_d = nc.dram_tensor("bp", [DIM], f32, kind="ExternalInput")
    lam_d = nc.dram_tensor("lam", [H], f32, kind="ExternalInput")
    out_d = nc.dram_tensor("out", [NQ, DIM], f32, kind="ExternalOutput")

    C = 6          # 768 / 128 feature chunks
    NPAIR = 6      # head pairs
    TT = 16        # token tiles of 128 in N
    QT = 4         # query sub-tiles of 128 in NQ

    with tile.TileContext(nc) as tc:
        with (
            tc.tile_pool(name="persist", bufs=1) as pp,
            tc.tile_pool(name="psum", bufs=1, space="PSUM") as psp,
        ):
            # ---- constants / small tiles ----
            ident = pp.tile([128, 128], f32, tag="ident")
            make_identity(nc, ident[:])
            lam_b = pp.tile([128, H], f32, tag="lam_b")
            nc.gpsimd.dma_start(
                out=lam_b[:],
                in_=bass.AP(tensor=lam_d, offset=0, ap=[[0, 128], [1, H]]),
            )
            bp_row = pp.tile([1, DIM], f32, tag="bp_row")
            nc.gpsimd.dma_start(
                out=bp_row[:],
                in_=bass.AP(tensor=bp_d, offset=0, ap=[[0, 1], [1, DIM]]),
            )
            ones_col = pp.tile([1, 128], f32, tag="ones_col")
            nc.vector.memset(ones_col[:], 1.0)

            # ---- persistent big tiles ----
            xT = pp.tile([128, C, N], f32r, tag="xT")
            xTb = pp.tile([128, C, N], bf16, tag="xTb")
            Y = pp.tile([128, QT, DIM], f32, tag="Y")
            yT = pp.tile([128, C, NQ], bf16, tag="yT")
            stats = pp.tile([128, QT, C + 1, 6], f32, tag="stats")
            wpj = pp.tile([128, C, DIM], bf16, tag="wpj")

            def dma_xT(lo, hi):
                nc.sync.dma_start(
                    xT[:, :, lo:hi],
                    xT_d[:, lo:hi].rearrange("(c p) m -> p c m", p=128),
                )

            def emit_xTb_slice(s):
                nc.vector.tensor_copy(
                    xTb[:, :, s * 512 : (s + 1) * 512],
                    xT[:, :, s * 512 : (s + 1) * 512],
                )

            with (
                tc.tile_pool(name="pairs", bufs=2) as wpool,
                tc.tile_pool(name="epool", bufs=int(os.environ.get("EPBUFS", "12"))) as ep,
            ):
                # ---- weight slice DMA + GEMM emit helpers ----
                def dma_wslice(tag, src_w, col0, dt=f32r):
                    t = wpool.tile([128, C, 128], dt, tag=tag, name=tag)
                    nc.sync.dma_start(
                        t[:],
                        src_w[:, col0 : col0 + 128].rearrange(
                            "(c p2) n -> p2 c n", p2=128
                        ),
                    )
                    return t

                def alloc_q(tag):
                    return wpool.tile([128, NQ], bf16, tag=tag, name=tag)

                def emit_q_fill(qp, wq, lo=0, hi=NQ):
                    ps = psp.tile([128, 512], f32, tag="mm", bufs=2, name="psq")
                    w = hi - lo
                    for c in range(C):
                        nc.tensor.matmul(
                            ps[:, 0:w],
                            wq[:, c, :],
                            xT[:, c, lo:hi],
                            start=(c == 0),
                            stop=(c == C - 1),
                        )
                    nc.vector.tensor_copy(qp[:, lo:hi], ps[:, 0:w])

                def alloc_k(tag):
                    return wpool.tile([128, N], bf16, tag=tag, name=tag)

                def emit_k_gemm(kt, wk, mt):
                    ps = psp.tile([128, 512], f32, tag="mm", bufs=2, name="psk")
                    for c in range(C):
                        nc.tensor.matmul(
                            ps[:],
                            wk[:, c, :],
                            xT[:, c, mt * 512 : (mt + 1) * 512],
                            start=(c == 0),
                            stop=(c == C - 1),
                        )
                    nc.vector.tensor_copy(kt[:, mt * 512 : (mt + 1) * 512], ps[:])

                def alloc_v1():
                    v = wpool.tile(
                        [128, TT, 2, HD + 1], bf16, tag="v1p", name="v1p"
                    )
                    nc.vector.memset(v[:, :, :, HD : HD + 1], 1.0)
                    return v

                def emit_v1_unit(vdst, wv, u):
                    # token tiles 2u, 2u+1; out token-major [tok, 2*HD]
                    ps = psp.tile([128, 512], f32, tag="mm", bufs=2, name="psv")
                    for half in range(2):
                        t = 2 * u + half
                        for c in range(C):
                            nc.tensor.matmul(
                                ps[:, half * 128 : (half + 1) * 128],
                                xTb[:, c, t * 128 : (t + 1) * 128],
                                wv[:, c, :],
                                start=(c == 0),
                                stop=(c == C - 1),
                            )
                    nc.vector.tensor_copy(
                        vdst[:, 2 * u : 2 * u + 2, :, 0:HD],
                        ps[:, 0:256].rearrange("p (t h d) -> p t h d", t=2, h=2),
                    )

                # ---- proj stage 1: accumulate chunks 0-4 of the proj
                # GEMM into an SBUF accumulator (runs as late deferred
                # units inside pair 5); chunk 5 joins at the tail ----
                pacc = pp.tile([128, QT, DIM], f32, tag="pacc")

                def emit_proj_stage1(j, half):
                    # chunks 0-4 of y@Wp, plus bias via rank-1 ones x bp
                    jr = slice(j * 128, (j + 1) * 128)
                    hs = slice(half * 384, (half + 1) * 384)
                    ps = psp.tile([128, 512], f32, tag="mm", bufs=2, name="ps1")
                    nc.tensor.matmul(
                        ps[:, 0:384],
                        ones_col[:, 0:128],
                        bp_row[:, hs],
                        start=True,
                        stop=False,
                    )
                    for c in range(5):
                        nc.tensor.matmul(
                            ps[:, 0:384],
                            yT[:, c, jr],
                            wpj[:, c, hs],
                            start=False,
                            stop=(c == 4),
                        )
                    nc.vector.tensor_copy(pacc[:, j, hs], ps[:, 0:384])

                # ---- per-head finish for chunk 5 (heads 10, 11) ----
                def emit_head_finish(h):
                    hh = h % 2
                    for j in range(QT):
                        tp = psp.tile([128, 512], f32, tag="mm", bufs=2, name="tp")
                        nc.tensor.transpose(
                            tp[0:64, 0:128],
                            Y[:, j, h * 64 : (h + 1) * 64],
                            ident[:],
                        )
                        nc.vector.tensor_copy(
                            yT[hh * 64 : (hh + 1) * 64, 5, j * 128 : (j + 1) * 128],
                            tp[0:64, 0:128],
                        )
                        nc.vector.bn_stats(
                            out=stats[:, j, 5 + hh, :],
                            in_=Y[:, j, h * 64 : (h + 1) * 64],
                        )

                # ---- chunk finish: transpose Y chunk + bn stats ----
                def emit_chunk_finish_j(cc, j):
                    tp = psp.tile([128, 512], f32, tag="mm", bufs=2, name="tp")
                    nc.tensor.transpose(
                        tp[:, 0:128],
                        Y[:, j, cc * 128 : (cc + 1) * 128],
                        ident[:],
                    )
                    nc.vector.tensor_copy(
                        yT[:, cc, j * 128 : (j + 1) * 128], tp[:, 0:128]
                    )
                    nc.vector.bn_stats(
                        out=stats[:, j, cc, :],
                        in_=Y[:, j, cc * 128 : (cc + 1) * 128],
                    )

                def emit_chunk_finish(cc):
                    for j in range(QT):
                        emit_chunk_finish_j(cc, j)

                # ---- one-shot work units + paced global queue ----
                # Every prefetched GEMM is a one-shot closure. The pair
                # that OWNS a unit force-runs it at the latest safe point
                # (deadline hooks inside its own chains); paced pops from
                # the global queue run units early when PE has slack, so
                # leftover work naturally flows into the last pairs where
                # the exp stream would otherwise throttle PE.
                def unit(fn):
                    st = [False]

                    def run():
                        if st[0]:
                            return False
                        st[0] = True
                        fn()
                        return True

                    return run

                global_q = deque()
                defer_q = deque()

                def pop1(_=None):
                    while global_q:
                        if global_q.popleft()():
                            return
                    while defer_q:
                        if defer_q.popleft()():
                            return

                def drain_all():
                    while global_q:
                        global_q.popleft()()
                    while defer_q:
                        defer_q.popleft()()

                PACE = int(os.environ.get("PACE", "2"))

                class PU:
                    pass

                def make_pair_units(wq1s, wk1s, wq2s, wk2s, wv1s,
                                    q1t, q2t, k1t, k2t, v1t, xtb=False):
                    pu = PU()
                    pu.q1 = unit(lambda: emit_q_fill(q1t, wq1s))

                    def k1fn(m):
                        def f():
                            emit_k_gemm(k1t, wk1s, m)
                            if xtb:
                                emit_xTb_slice(m)

                        return f

                    pu.k1 = [unit(k1fn(m)) for m in range(4)]
                    pu.v1 = [
                        unit(lambda u=u: emit_v1_unit(v1t, wv1s, u))
                        for u in range(8)
                    ]
                    pu.q2 = unit(lambda: emit_q_fill(q2t, wq2s))
                    pu.k2 = [
                        unit(lambda m=m: emit_k_gemm(k2t, wk2s, m))
                        for m in range(4)
                    ]
                    return pu

                def c1_qk_hook(pu):
                    def h(g):
                        pu.q1()
                        for m in range(g // 2 + 1):
                            pu.k1[m]()
                        if g >= 2:
                            pu.v1[g - 2]()
                        if qk_pop_ok(g):
                            pop1()

                    return h

                def c1_av_hook(pu):
                    def h(j):
                        if j == 0:
                            for u in pu.v1:
                                u()
                        else:
                            pop1()

                    return h

                def c2_qk_hook(pu):
                    def h(g):
                        pu.q2()
                        for m in range(g // 2 + 1):
                            pu.k2[m]()
                        if g % PACE == 1:
                            pop1()

                    return h

                QKPOP = os.environ.get("QKPOP", "none")

                def qk_pop_ok(g):
                    if QKPOP == "none":
                        return False
                    if QKPOP == "g7":
                        return g == 7
                    if QKPOP == "odd":
                        return g % 2 == 1
                    return True

                DEFPOP = int(os.environ.get("DEFPOP", "1"))

                def paced_qk(g):
                    if qk_pop_ok(g):
                        pop1()
                    elif DEFPOP and defer_q and not global_q:
                        pop1()

                AVJ = int(os.environ.get("AVJ", "1"))

                def paced_av(j):
                    if j >= AVJ:
                        pop1()

                # ---- attention chain ----
                def attn_chain(hh, kt, qp, v1p, qk_hook=None, av_hook=None):
                    po = hh * HD
                    av = psp.tile(
                        [128, QT, HD + 1], f32, tag="av", bufs=2, name="av"
                    )
                    ets = []
                    for g in range(8):
                        if qk_hook is not None:
                            qk_hook(g)
                        qk = psp.tile(
                            [128, 2, 512], f32, tag="qk", bufs=2, name="qk"
                        )
                        for g2 in range(2):
                            mc = g * 2 + g2
                            nc.tensor.matmul(
                                qk[:, g2, :],
                                kt[po : po + 64, mc * 128 : (mc + 1) * 128],
                                qp[po : po + 64, :],
                                start=True,
                                stop=True,
                            )
                        e_t = ep.tile([128, 2, 512], bf16, tag="E", name="e_t")
                        nc.scalar.activation(
                            e_t[:],
                            qk[:],
                            mybir.ActivationFunctionType.Exp,
                            scale=0.125,
                        )
                        ets.append(e_t)
                    for j in range(QT):
                        if av_hook is not None:
                            av_hook(j)
                        for mc in range(16):
                            nc.tensor.matmul(
                                av[:, j, :],
                                ets[mc // 2][:, mc % 2, j * 128 : (j + 1) * 128],
                                v1p[:, mc, hh, :],
                                start=(mc == 0),
                                stop=(mc == 15),
                            )
                    return av

                def combine(h, av1, av2):
                    # reads the AV psums directly; av1 released first so
                    # the next chain can reuse the psum buffer sooner
                    r1 = wpool.tile([128, QT, 1], f32, tag="r1", bufs=2, name="r1")
                    nc.vector.reciprocal(r1[:], av1[:, :, HD : HD + 1])
                    t1 = wpool.tile([128, QT, HD], f32, tag="t1", bufs=1, name="t1")
                    r1b = bass.AP(
                        tensor=r1.tensor,
                        offset=r1.offset,
                        ap=[r1.ap[0], r1.ap[1], [0, HD]],
                    )
                    nc.vector.tensor_tensor(
                        out=t1[:],
                        in0=av1[:, :, 0:HD],
                        in1=r1b,
                        op=mybir.AluOpType.mult,
                    )
                    r2 = wpool.tile([128, QT, 1], f32, tag="r2", bufs=2, name="r2")
                    nc.vector.reciprocal(r2[:], av2[:, :, HD : HD + 1])
                    lam_h = lam_b[:, h : h + 1]
                    lam_bc = bass.AP(
                        tensor=lam_h.tensor,
                        offset=lam_h.offset,
                        ap=[lam_h.ap[0], [0, QT], [0, 1]],
                    )
                    nc.vector.tensor_tensor(
                        out=r2[:], in0=r2[:], in1=lam_bc, op=mybir.AluOpType.mult
                    )
                    t2 = wpool.tile([128, QT, HD], f32, tag="t2", bufs=1, name="t2")
                    r2b = bass.AP(
                        tensor=r2.tensor,
                        offset=r2.offset,
                        ap=[r2.ap[0], r2.ap[1], [0, HD]],
                    )
                    nc.vector.tensor_tensor(
                        out=t2[:],
                        in0=av2[:, :, 0:HD],
                        in1=r2b,
                        op=mybir.AluOpType.mult,
                    )
                    nc.vector.tensor_tensor(
                        out=Y[:, :, h * 64 : (h + 1) * 64],
                        in0=t1[:],
                        in1=t2[:],
                        op=mybir.AluOpType.subtract,
                    )

                # ---- startup DMAs (issue order = DMA device order) ----
                dma_xT(0, 256)
                wq1 = dma_wslice("wq1", w1_d, 0)
                dma_xT(256, 512)
                wk1 = dma_wslice("wk1", w1_d, DIM)
                dma_xT(512, 1024)
                wv1 = dma_wslice("wv1", wv1_d, 0, bf16)
                dma_xT(1024, 1536)
                dma_xT(1536, 2048)
                wq2 = dma_wslice("wq2", w2_d, 0)
                wk2 = dma_wslice("wk2", w2_d, DIM)

                # ---- pair-0 prologue ----
                q1p = alloc_q("q1p")
                q2p = alloc_q("q2p")
                emit_q_fill(q1p, wq1, 0, 256)
                emit_q_fill(q1p, wq1, 256, 512)
                k1T = alloc_k("k1T")
                k2T = alloc_k("k2T")
                v1p = alloc_v1()
                pu = make_pair_units(wq1, wk1, wq2, wk2, wv1,
                                     q1p, q2p, k1T, k2T, v1p, xtb=True)
                pu.q1()  # already filled above; mark consumed
                pu.k1[0]()

                # ---- pair loop ----
                for p in range(NPAIR):
                    nx = p + 1
                    if nx < NPAIR:
                        wq1n = dma_wslice("wq1", w1_d, nx * 128)
                        wk1n = dma_wslice("wk1", w1_d, DIM + nx * 128)
                        wq2n = dma_wslice("wq2", w2_d, nx * 128)
                        wk2n = dma_wslice("wk2", w2_d, DIM + nx * 128)
                        wv1n = dma_wslice("wv1", wv1_d, nx * 128, bf16)
                        if nx == 4:
                            nc.sync.dma_start(
                                wpj[:],
                                wp_d[:, :].rearrange("(c p) n -> p c n", p=128),
                            )
                        q1pn = alloc_q("q1p")
                        q2pn = alloc_q("q2p")
                        k1Tn = alloc_k("k1T")
                        k2Tn = alloc_k("k2T")
                        v1pn = alloc_v1()
                        pun = make_pair_units(wq1n, wk1n, wq2n, wk2n, wv1n,
                                              q1pn, q2pn, k1Tn, k2Tn, v1pn)
                        global_q.append(pun.q1)
                        for u in pun.k1:
                            global_q.append(u)
                        for u in pun.v1:
                            global_q.append(u)
                        global_q.append(pun.q2)
                        for u in pun.k2:
                            global_q.append(u)
                        if p >= 1:
                            defer_q.append(
                                unit(lambda cc=p - 1: emit_chunk_finish(cc))
                            )
                    elif p == NPAIR - 1:
                        defer_q.append(unit(lambda: emit_chunk_finish(4)))
                        for j in range(QT):
                            for half in range(2):
                                defer_q.append(
                                    unit(
                                        lambda jj=j, hh=half: emit_proj_stage1(
                                            jj, hh
                                        )
                                    )
                                )

                    av1 = attn_chain(0, k1T, q1p, v1p,
                                     qk_hook=c1_qk_hook(pu),
                                     av_hook=c1_av_hook(pu))
                    av2 = attn_chain(0, k2T, q2p, v1p,
                                     qk_hook=c2_qk_hook(pu),
                                     av_hook=paced_av)
                    combine(2 * p, av1, av2)
                    if p == NPAIR - 1:
                        defer_q.append(unit(lambda: emit_head_finish(10)))
                    av1 = attn_chain(1, k1T, q1p, v1p,
                                     qk_hook=paced_qk, av_hook=paced_av)
                    av2 = attn_chain(1, k2T, q2p, v1p,
                                     qk_hook=paced_qk, av_hook=paced_av)
                    combine(2 * p + 1, av1, av2)
                    if p == NPAIR - 1:
                        emit_head_finish(11)

                    if nx < NPAIR:
                        k1T, k2T, v1p = k1Tn, k2Tn, v1pn
                        q1p, q2p = q1pn, q2pn
                        pu = pun

                drain_all()

            # ---- tail: per query tile: chunk-5 finish -> rms -> proj ----
            with tc.tile_pool(name="proj", bufs=1) as prj:
                mv = prj.tile([128, QT, 2], f32, tag="mv")
                rms = prj.tile([128, QT], f32, tag="rms")
                eps_t = prj.tile([128, 1], f32, tag="eps_t")
                nc.vector.memset(eps_t[:], EPS)

                def emit_rms_j(j):
                    nc.vector.bn_aggr(out=mv[:, j, :], in_=stats[:, j])

                    nc.vector.tensor_tensor(
                        out=mv[:, j, 0:1],
                        in0=mv[:, j, 0:1],
                        in1=mv[:, j, 0:1],
                        op=mybir.AluOpType.mult,
                    )
                    nc.vector.tensor_tensor(
                        out=mv[:, j, 1:2],
                        in0=mv[:, j, 1:2],
                        in1=mv[:, j, 0:1],
                        op=mybir.AluOpType.add,
                    )
                    nc.scalar.activation(
                        rms[:, j : j + 1],
                        mv[:, j, 1:2],
                        mybir.ActivationFunctionType.Sqrt,
                        bias=eps_t[:],
                        scale=1.0,
                    )
                    nc.vector.reciprocal(rms[:, j : j + 1], rms[:, j : j + 1])

                def emit_proj_j(j):
                    # stage 2: chunk-5 contribution joins the stage-1
                    # accumulator; rms folds into the epilogue:
                    # (y*rms) @ Wp = (y @ Wp) * rms  (rms is per-token)
                    jr = slice(j * 128, (j + 1) * 128)
                    osb2 = prj.tile(
                        [128, DIM], f32, tag="out_sb", bufs=2, name="osb2"
                    )
                    osb3 = prj.tile(
                        [128, DIM], f32, tag="out_sb3", bufs=2, name="osb3"
                    )
                    for half in range(2):
                        ps = psp.tile([128, 512], f32, tag="mm", bufs=2, name="psp2")
                        nc.tensor.matmul(
                            ps[:, 0:384],
                            yT[:, 5, jr],
                            wpj[:, 5, half * 384 : (half + 1) * 384],
                            start=True,
                            stop=True,
                        )
                        hs = slice(half * 384, (half + 1) * 384)
                        nc.vector.tensor_tensor(
                            out=osb2[:, hs],
                            in0=pacc[:, j, hs],
                            in1=ps[:, 0:384],
                            op=mybir.AluOpType.add,
                        )
                        nc.scalar.activation(
                            osb3[:, hs],
                            osb2[:, hs],
                            mybir.ActivationFunctionType.Copy,
                            scale=rms[:, j : j + 1],
                        )
                        nc.sync.dma_start(
                            out_d[j * 128 : (j + 1) * 128, hs], osb3[:, hs]
                        )

                for j in range(QT):
                    emit_rms_j(j)
                    emit_proj_j(j)

    _split_waits(nc)
    return nc


def kernel(x, W_qkv1, W_qkv2, W_proj, b_proj, norm_w, lambda_1, lambda_2, xpos):
    import ml_dtypes
    from concourse.bass_utils import run_bass_kernel_spmd

    if "nc" not in _cache:
        _cache["nc"] = _build()
    nc = _cache["nc"]

    bf16 = ml_dtypes.bfloat16
    x = np.asarray(x, dtype=np.float32)
    w1 = np.asarray(W_qkv1, dtype=np.float32)
    w2 = np.asarray(W_qkv2, dtype=np.float32)
    w1qk = np.ascontiguousarray(w1[:, : 2 * DIM])
    w2qk = np.ascontiguousarray(w2[:, : 2 * DIM])
    wv1b = np.ascontiguousarray(w1[:, 2 * DIM :]).astype(bf16)
    wpb = np.ascontiguousarray(
        np.asarray(norm_w, dtype=np.float32)[:, None]
        * np.asarray(W_proj, dtype=np.float32)
    ).astype(bf16)
    bp = np.ascontiguousarray(np.asarray(b_proj, dtype=np.float32))
    lam = np.ascontiguousarray(
        (
            np.asarray(lambda_1, dtype=np.float32)
            - np.asarray(lambda_2, dtype=np.float32)
            + LAMBDA_INIT
        ).astype(np.float32)
    )

    in_maps = []
    for c in range(NCORES):
        b, qi = c // 4, c % 4
        # rotate tokens so this core's query block comes first; attention
        # is permutation-invariant over keys so only q/out order matters
        xr = np.roll(x[b], -qi * NQ, axis=0)
        in_maps.append(
            {
                "xTp": np.ascontiguousarray(xr.T),
                "w1": w1qk,
                "w2": w2qk,
                "wv1b": wv1b,
                "wp": wpb,
                "bp": bp,
                "lam": lam,
            }
        )

    res = run_bass_kernel_spmd(nc, in_maps, core_ids=list(range(NCORES)))
    out = np.empty((B, N, DIM), dtype=np.float32)
    for c in range(NCORES):
        b, qi = c // 4, c % 4
        out[b, qi * NQ : (qi + 1) * NQ, :] = res.results[c]["out"]
    return out



# revision 18
# speedup vs baseline: 1.0805x; 1.0805x over previous
"""DIFF-Attention Trainium2 kernel (v4: fp8 DoubleRow attn2 path).

Problem: B=2, N=2048, DIM=768, H=12, HD=64, two qkv projections, two
softmax attention maps, diff = attn1 - lam*attn2, out = diff @ v1,
RMSNorm, proj.

v4: the attn2 branch enters the output scaled by lam = l1-l2+0.1 ~=
0.108, so its quantization noise is suppressed ~9x. Exploit with
fp8e4m3 + MatmulPerfMode.DoubleRow (2 contraction rows packed per
matmul at 0.5 cycles/row) on: the k2/q2 qkv GEMMs (host-cast x8/w28
inputs), and AV2 (E2 = exp(S2*0.125 - ESH) written fp8 by the
activation, v18 = fp8 copy of v1). ESH shifts exp output under
e4m3's +-240 range (softmax is shift-invariant). QK2 stays bf16:
feeding it fp8 would need a [32,2,free] relayout that costs more
DVE than the PE it saves.

Sharding: 8 cores; core c handles batch b = c//4 and query tokens
[512*(c%4), 512*(c%4)+512). Attention is permutation-invariant over
keys, so the host hands each core x^T with the token axis ROTATED so
that the core's 512 query tokens come first: q GEMMs read the first
512 columns of the same xT tile the k/v GEMMs read (no separate xq
DMA). Each core computes k1/k2/v1 for the whole (permuted) batch and
q/attention/norm/proj only for its 512 query tokens. No collectives.

Structure:
  - Each attention chain runs its 8 QK+exp groups first (8 E tiles
    buffered), then 4 query-tile AV passes with out[query, hd]
    (queries on psum partitions, 65-wide moving dim). Sequential
    per-region psum accumulation — interleaved start/stop groups
    within one psum bank silently corrupt results.
  - combine() reads the AV psums directly (no PE transpose of O).
  - v1 is computed per head-pair (prefetched like k/q) as a bf16
    token-major GEMM from an on-chip bf16 copy of x^T.
  - Work units (next-pair k/q/v GEMMs, prev-chunk Y transpose+stats)
    are popped one per attention group. Units only needed by LATER
    chains (q2/k2 fills, chunk finish) are carryable across the pair
    boundary so the last pairs keep PE fed.
  - Tail pipelines per query tile: Y-chunk-5 transpose -> rms -> proj
    -> epilogue -> half-row output DMAs.

Numerics: f32r (tf32-like) qkv/QK GEMMs, bf16 exp(S) / AV / v1 / proj,
fp32 PSUM accumulation, RMSNorm stats in fp32.
"""

import os
import numpy as np

B, N, DIM, H, HD = 2, 2048, 768, 12, 64
NQ = 512            # query tokens per core
LAMBDA_INIT = 0.1
EPS = 1e-6
NCORES = 8
ESH = 5.0           # exp shift for fp8 E2 (max observed logit ~8.7; e4m3 max 240)

_cache = {}


def _split_waits(nc, max_waits=1):
    """The walrus build in this environment rejects instructions carrying
    more than one explicit sync wait. Hoist excess waits onto NoOps
    inserted just before, on the same engine (same-engine program order
    makes this semantically equivalent)."""
    import concourse.mybir as mybir

    ctr = 0
    for f in nc.m.functions:
        for b in f.blocks:
            out = []
            changed = False
            for inst in b.instructions:
                si = inst.sync_info
                waits = list(si.on_wait) if si is not None and si.on_wait else []
                if len(waits) > max_waits:
                    changed = True
                    keep = waits[-max_waits:]
                    excess = waits[:-max_waits]
                    for i in range(0, len(excess), max_waits):
                        ctr += 1
                        nop = mybir.InstNoOp(
                            name=f"I-waitsplit-{ctr}", ins=[], outs=[]
                        )
                        nop.engine = inst.engine
                        nop.sync_info = mybir.SyncInfo(
                            on_wait=excess[i : i + max_waits], on_update=[]
                        )
                        out.append(nop)
                    inst.sync_info = mybir.SyncInfo(
                        on_wait=keep,
                        on_update=list(si.on_update) if si.on_update else [],
                    )
                out.append(inst)
            if changed:
                b.instructions = out


def _build():
    import concourse.bass as bass
    import concourse.mybir as mybir
    import concourse.tile as tile
    from concourse.masks import make_identity
    from collections import deque

    f32 = mybir.dt.float32
    f32r = mybir.dt.float32r
    bf16 = mybir.dt.bfloat16
    fp8 = mybir.dt.float8e4
    DR = mybir.MatmulPerfMode.DoubleRow

    nc = bass.Bass(trn_type="TRN2")

    xT_d = nc.dram_tensor("xTp", [DIM, N], f32r, kind="ExternalInput")
    x8_d = nc.dram_tensor("x8", [DIM, N], fp8, kind="ExternalInput")
    w1_d = nc.dram_tensor("w1", [DIM, 2 * DIM], f32r, kind="ExternalInput")
    w28_d = nc.dram_tensor("w28", [DIM, 2 * DIM], fp8, kind="ExternalInput")
    wv1_d = nc.dram_tensor("wv1b", [DIM, DIM], bf16, kind="ExternalInput")
    wp_d = nc.dram_tensor("wp", [DIM, DIM], bf16, kind="ExternalInput")
    bp_d = nc.dram_tensor("bp", [DIM], f32, kind="ExternalInput")
    lam_d = nc.dram_tensor("lam", [H], f32, kind="ExternalInput")
    out_d = nc.dram_tensor("out", [NQ, DIM], f32, kind="ExternalOutput")

    C = 6          # 768 / 128 feature chunks
    NPAIR = 6      # head pairs
    TT = 16        # token tiles of 128 in N
    QT = 4         # query sub-tiles of 128 in NQ

    with tile.TileContext(nc) as tc:
        with (
            tc.tile_pool(name="persist", bufs=1) as pp,
            tc.tile_pool(name="psum", bufs=1, space="PSUM") as psp,
        ):
            # ---- constants / small tiles ----
            ident = pp.tile([128, 128], f32, tag="ident")
            make_identity(nc, ident[:])
            lam_b = pp.tile([128, H], f32, tag="lam_b")
            nc.gpsimd.dma_start(
                out=lam_b[:],
                in_=bass.AP(tensor=lam_d, offset=0, ap=[[0, 128], [1, H]]),
            )
            bp_row = pp.tile([1, DIM], f32, tag="bp_row")
            nc.gpsimd.dma_start(
                out=bp_row[:],
                in_=bass.AP(tensor=bp_d, offset=0, ap=[[0, 1], [1, DIM]]),
            )
            ones_col = pp.tile([1, 128], f32, tag="ones_col")
            nc.vector.memset(ones_col[:], 1.0)
            esh_t = pp.tile([128, 1], f32, tag="esh_t")
            nc.vector.memset(esh_t[:], -ESH)

            # ---- persistent big tiles ----
            xT = pp.tile([128, C, N], f32r, tag="xT")
            xT8 = pp.tile([128, C, N], fp8, tag="xT8")
            xTb = pp.tile([128, C, N], bf16, tag="xTb")
            Y = pp.tile([128, QT, DIM], f32, tag="Y")
            yT = pp.tile([128, C, NQ], bf16, tag="yT")
            stats = pp.tile([128, QT, C + 1, 6], f32, tag="stats")
            wpj = pp.tile([128, C, DIM], bf16, tag="wpj")

            def dma_xT(lo, hi):
                nc.sync.dma_start(
                    xT[:, :, lo:hi],
                    xT_d[:, lo:hi].rearrange("(c p) m -> p c m", p=128),
                )

            def dma_xT8(lo, hi):
                nc.sync.dma_start(
                    xT8[:, :, lo:hi],
                    x8_d[:, lo:hi].rearrange("(c p) m -> p c m", p=128),
                )

            def emit_xTb_slice(s):
                nc.vector.tensor_copy(
                    xTb[:, :, s * 512 : (s + 1) * 512],
                    xT[:, :, s * 512 : (s + 1) * 512],
                )

            with (
                tc.tile_pool(name="pairs", bufs=2) as wpool,
                tc.tile_pool(name="epool", bufs=int(os.environ.get("EPBUFS", "9"))) as ep,
            ):
                # ---- weight slice DMA + GEMM emit helpers ----
                def dma_wslice(tag, src_w, col0, dt=f32r):
                    t = wpool.tile([128, C, 128], dt, tag=tag, name=tag)
                    nc.sync.dma_start(
                        t[:],
                        src_w[:, col0 : col0 + 128].rearrange(
                            "(c p2) n -> p2 c n", p2=128
                        ),
                    )
                    return t

                def dma_w8slice(tag, col0):
                    # fp8 DoubleRow layout: [part, chunk-pair, row-half, col]
                    t = wpool.tile([128, 3, 2, 128], fp8, tag=tag, name=tag)
                    nc.sync.dma_start(
                        t[:],
                        w28_d[:, col0 : col0 + 128].rearrange(
                            "(cc two p2) n -> p2 cc two n", p2=128, two=2
                        ),
                    )
                    return t

                def alloc_q(tag):
                    return wpool.tile([128, NQ], bf16, tag=tag, name=tag)

                def emit_q_fill(qp, wq, lo=0, hi=NQ):
                    ps = psp.tile([128, 512], f32, tag="mm", bufs=2, name="psq")
                    w = hi - lo
                    for c in range(C):
                        nc.tensor.matmul(
                            ps[:, 0:w],
                            wq[:, c, :],
                            xT[:, c, lo:hi],
                            start=(c == 0),
                            stop=(c == C - 1),
                        )
                    nc.vector.tensor_copy(qp[:, lo:hi], ps[:, 0:w])

                def alloc_k(tag):
                    return wpool.tile([128, N], bf16, tag=tag, name=tag)

                def emit_k_gemm(kt, wk, mt):
                    ps = psp.tile([128, 512], f32, tag="mm", bufs=2, name="psk")
                    for c in range(C):
                        nc.tensor.matmul(
                            ps[:],
                            wk[:, c, :],
                            xT[:, c, mt * 512 : (mt + 1) * 512],
                            start=(c == 0),
                            stop=(c == C - 1),
                        )
                    nc.vector.tensor_copy(kt[:, mt * 512 : (mt + 1) * 512], ps[:])

                def emit_q_fill8(qp, wq8, lo=0, hi=NQ):
                    ps = psp.tile([128, 512], f32, tag="mm", bufs=2, name="psq8")
                    w = hi - lo
                    for cc in range(3):
                        nc.tensor.matmul(
                            ps[:, 0:w],
                            wq8[:, cc],
                            xT8[:, 2 * cc : 2 * cc + 2, lo:hi],
                            start=(cc == 0),
                            stop=(cc == 2),
                            perf_mode=DR,
                        )
                    nc.vector.tensor_copy(qp[:, lo:hi], ps[:, 0:w])

                def emit_k_gemm8(kt, wk8, mt):
                    ps = psp.tile([128, 512], f32, tag="mm", bufs=2, name="psk8")
                    for cc in range(3):
                        nc.tensor.matmul(
                            ps[:],
                            wk8[:, cc],
                            xT8[:, 2 * cc : 2 * cc + 2, mt * 512 : (mt + 1) * 512],
                            start=(cc == 0),
                            stop=(cc == 2),
                            perf_mode=DR,
                        )
                    nc.vector.tensor_copy(kt[:, mt * 512 : (mt + 1) * 512], ps[:])

                def alloc_v1():
                    v = wpool.tile(
                        [128, TT, 2, HD + 1], bf16, tag="v1p", name="v1p"
                    )
                    nc.vector.memset(v[:, :, :, HD : HD + 1], 1.0)
                    return v

                def alloc_v18():
                    v = wpool.tile(
                        [128, TT, 2, HD + 1], fp8, tag="v18", name="v18"
                    )
                    nc.vector.memset(v[:, :, :, HD : HD + 1], 1.0)
                    return v

                def emit_v1_unit(vdst, v8dst, wv, u):
                    # token tiles 2u, 2u+1; out token-major [tok, 2*HD]
                    ps = psp.tile([128, 512], f32, tag="mm", bufs=2, name="psv")
                    for half in range(2):
                        t = 2 * u + half
                        for c in range(C):
                            nc.tensor.matmul(
                                ps[:, half * 128 : (half + 1) * 128],
                                xTb[:, c, t * 128 : (t + 1) * 128],
                                wv[:, c, :],
                                start=(c == 0),
                                stop=(c == C - 1),
                            )
                    nc.vector.tensor_copy(
                        vdst[:, 2 * u : 2 * u + 2, :, 0:HD],
                        ps[:, 0:256].rearrange("p (t h d) -> p t h d", t=2, h=2),
                    )
                    nc.vector.tensor_copy(
                        v8dst[:, 2 * u : 2 * u + 2, :, 0:HD],
                        ps[:, 0:256].rearrange("p (t h d) -> p t h d", t=2, h=2),
                    )

                # ---- proj stage 1: accumulate chunks 0-4 of the proj
                # GEMM into an SBUF accumulator (runs as late deferred
                # units inside pair 5); chunk 5 joins at the tail ----
                pacc = pp.tile([128, QT, DIM], f32, tag="pacc")

                def emit_proj_stage1(j, half):
                    # chunks 0-4 of y@Wp, plus bias via rank-1 ones x bp
                    jr = slice(j * 128, (j + 1) * 128)
                    hs = slice(half * 384, (half + 1) * 384)
                    ps = psp.tile([128, 512], f32, tag="mm", bufs=2, name="ps1")
                    nc.tensor.matmul(
                        ps[:, 0:384],
                        ones_col[:, 0:128],
                        bp_row[:, hs],
                        start=True,
                        stop=False,
                    )
                    for c in range(5):
                        nc.tensor.matmul(
                            ps[:, 0:384],
                            yT[:, c, jr],
                            wpj[:, c, hs],
                            start=False,
                            stop=(c == 4),
                        )
                    nc.vector.tensor_copy(pacc[:, j, hs], ps[:, 0:384])

                # ---- per-head finish for chunk 5 (heads 10, 11) ----
                def emit_head_finish_j(h, j):
                    hh = h % 2
                    tp = psp.tile([128, 512], f32, tag="mm", bufs=2, name="tp")
                    nc.tensor.transpose(
                        tp[0:64, 0:128],
                        Y[:, j, h * 64 : (h + 1) * 64],
                        ident[:],
                    )
                    nc.vector.tensor_copy(
                        yT[hh * 64 : (hh + 1) * 64, 5, j * 128 : (j + 1) * 128],
                        tp[0:64, 0:128],
                    )
                    nc.vector.bn_stats(
                        out=stats[:, j, 5 + hh, :],
                        in_=Y[:, j, h * 64 : (h + 1) * 64],
                    )

                def emit_head_finish(h):
                    for j in range(QT):
                        emit_head_finish_j(h, j)

                # ---- chunk finish: transpose Y chunk + bn stats ----
                def emit_chunk_finish_j(cc, j):
                    tp = psp.tile([128, 512], f32, tag="mm", bufs=2, name="tp")
                    nc.tensor.transpose(
                        tp[:, 0:128],
                        Y[:, j, cc * 128 : (cc + 1) * 128],
                        ident[:],
                    )
                    nc.vector.tensor_copy(
                        yT[:, cc, j * 128 : (j + 1) * 128], tp[:, 0:128]
                    )
                    nc.vector.bn_stats(
                        out=stats[:, j, cc, :],
                        in_=Y[:, j, cc * 128 : (cc + 1) * 128],
                    )

                def emit_chunk_finish(cc):
                    for j in range(QT):
                        emit_chunk_finish_j(cc, j)

                # ---- one-shot work units + paced global queue ----
                # Every prefetched GEMM is a one-shot closure. The pair
                # that OWNS a unit force-runs it at the latest safe point
                # (deadline hooks inside its own chains); paced pops from
                # the global queue run units early when PE has slack, so
                # leftover work naturally flows into the last pairs where
                # the exp stream would otherwise throttle PE.
                def unit(fn):
                    st = [False]

                    def run():
                        if st[0]:
                            return False
                        st[0] = True
                        fn()
                        return True

                    return run

                global_q = deque()
                defer_q = deque()

                def pop1(_=None):
                    while global_q:
                        if global_q.popleft()():
                            return
                    while defer_q:
                        if defer_q.popleft()():
                            return

                def drain_all():
                    while global_q:
                        global_q.popleft()()
                    while defer_q:
                        defer_q.popleft()()

                PACE = int(os.environ.get("PACE", "2"))

                class PU:
                    pass

                def make_pair_units(wq1s, wk1s, wq2s8, wk2s8, wv1s,
                                    q1t, q2t, k1t, k2t, v1t, v8t, xtb=False):
                    pu = PU()
                    pu.q1 = unit(lambda: emit_q_fill(q1t, wq1s))

                    def k1fn(m):
                        def f():
                            emit_k_gemm(k1t, wk1s, m)
                            if xtb:
                                emit_xTb_slice(m)

                        return f

                    pu.k1 = [unit(k1fn(m)) for m in range(4)]
                    pu.v1 = [
                        unit(lambda u=u: emit_v1_unit(v1t, v8t, wv1s, u))
                        for u in range(8)
                    ]
                    pu.q2 = unit(lambda: emit_q_fill8(q2t, wq2s8))
                    pu.k2 = [
                        unit(lambda m=m: emit_k_gemm8(k2t, wk2s8, m))
                        for m in range(4)
                    ]
                    return pu

                def c1_qk_hook(pu):
                    def h(g):
                        pu.q1()
                        for m in range(g // 2 + 1):
                            pu.k1[m]()
                        if g >= 2:
                            pu.v1[g - 2]()
                        if qk_pop_ok(g):
                            pop1()

                    return h

                def c1_av_hook(pu):
                    def h(j):
                        if j == 0:
                            for u in pu.v1:
                                u()
                        else:
                            pop1()

                    return h

                def c2_qk_hook(pu):
                    def h(g):
                        pu.q2()
                        for m in range(g // 2 + 1):
                            pu.k2[m]()
                        if g % PACE == 1:
                            pop1()

                    return h

                QKPOP = os.environ.get("QKPOP", "none")

                def qk_pop_ok(g):
                    if QKPOP == "none":
                        return False
                    if QKPOP == "g7":
                        return g == 7
                    if QKPOP == "odd":
                        return g % 2 == 1
                    return True

                DEFPOP = int(os.environ.get("DEFPOP", "1"))

                def paced_qk(g):
                    if qk_pop_ok(g):
                        pop1()
                    elif DEFPOP and defer_q and not global_q:
                        pop1()

                AVJ = int(os.environ.get("AVJ", "1"))

                def paced_av(j):
                    if j >= AVJ:
                        pop1()

                # ---- attention chain ----
                def attn_chain(hh, kt, qp, v1p, qk_hook=None, av_hook=None,
                               fp8av=False):
                    po = hh * HD
                    av = psp.tile(
                        [128, QT, HD + 1], f32, tag="av", bufs=2, name="av"
                    )
                    ets = []
                    for g in range(8):
                        if qk_hook is not None:
                            qk_hook(g)
                        qk = psp.tile(
                            [128, 2, 512], f32, tag="qk", bufs=2, name="qk"
                        )
                        for g2 in range(2):
                            mc = g * 2 + g2
                            nc.tensor.matmul(
                                qk[:, g2, :],
                                kt[po : po + 64, mc * 128 : (mc + 1) * 128],
                                qp[po : po + 64, :],
                                start=True,
                                stop=True,
                            )
                        if fp8av:
                            e_t = ep.tile(
                                [128, 2, 512], fp8, tag="E8", name="e8_t"
                            )
                            nc.scalar.activation(
                                e_t[:],
                                qk[:],
                                mybir.ActivationFunctionType.Exp,
                                bias=esh_t[:],
                                scale=0.125,
                            )
                        else:
                            e_t = ep.tile(
                                [128, 2, 512], bf16, tag="E", name="e_t"
                            )
                            nc.scalar.activation(
                                e_t[:],
                                qk[:],
                                mybir.ActivationFunctionType.Exp,
                                scale=0.125,
                            )
                        ets.append(e_t)
                    for j in range(QT):
                        if av_hook is not None:
                            av_hook(j)
                        if fp8av:
                            for g in range(8):
                                nc.tensor.matmul(
                                    av[:, j, :],
                                    ets[g][:, :, j * 128 : (j + 1) * 128],
                                    v1p[:, 2 * g : 2 * g + 2, hh, :],
                                    start=(g == 0),
                                    stop=(g == 7),
                                    perf_mode=DR,
                                )
                        else:
                            for mc in range(16):
                                nc.tensor.matmul(
                                    av[:, j, :],
                                    ets[mc // 2][:, mc % 2, j * 128 : (j + 1) * 128],
                                    v1p[:, mc, hh, :],
                                    start=(mc == 0),
                                    stop=(mc == 15),
                                )
                    return av

                def combine(h, av1, av2):
                    # reads the AV psums directly; av1 released first so
                    # the next chain can reuse the psum buffer sooner
                    r1 = wpool.tile([128, QT, 1], f32, tag="r1", bufs=2, name="r1")
                    nc.vector.reciprocal(r1[:], av1[:, :, HD : HD + 1])
                    t1 = wpool.tile([128, QT, HD], f32, tag="t1", bufs=1, name="t1")
                    r1b = bass.AP(
                        tensor=r1.tensor,
                        offset=r1.offset,
                        ap=[r1.ap[0], r1.ap[1], [0, HD]],
                    )
                    nc.vector.tensor_tensor(
                        out=t1[:],
                        in0=av1[:, :, 0:HD],
                        in1=r1b,
                        op=mybir.AluOpType.mult,
                    )
                    r2 = wpool.tile([128, QT, 1], f32, tag="r2", bufs=2, name="r2")
                    nc.vector.reciprocal(r2[:], av2[:, :, HD : HD + 1])
                    lam_h = lam_b[:, h : h + 1]
                    lam_bc = bass.AP(
                        tensor=lam_h.tensor,
                        offset=lam_h.offset,
                        ap=[lam_h.ap[0], [0, QT], [0, 1]],
                    )
                    nc.vector.tensor_tensor(
                        out=r2[:], in0=r2[:], in1=lam_bc, op=mybir.AluOpType.mult
                    )
                    t2 = wpool.tile([128, QT, HD], f32, tag="t2", bufs=1, name="t2")
                    r2b = bass.AP(
                        tensor=r2.tensor,
                        offset=r2.offset,
                        ap=[r2.ap[0], r2.ap[1], [0, HD]],
                    )
                    nc.vector.tensor_tensor(
                        out=t2[:],
                        in0=av2[:, :, 0:HD],
                        in1=r2b,
                        op=mybir.AluOpType.mult,
                    )
                    nc.vector.tensor_tensor(
                        out=Y[:, :, h * 64 : (h + 1) * 64],
                        in0=t1[:],
                        in1=t2[:],
                        op=mybir.AluOpType.subtract,
                    )

                # ---- startup DMAs (issue order = DMA device order) ----
                dma_xT(0, 256)
                wq1 = dma_wslice("wq1", w1_d, 0)
                dma_xT(256, 512)
                wk1 = dma_wslice("wk1", w1_d, DIM)
                dma_xT(512, 1024)
                wv1 = dma_wslice("wv1", wv1_d, 0, bf16)
                dma_xT(1024, 1536)
                dma_xT(1536, 2048)
                dma_xT8(0, 1024)
                wq2 = dma_w8slice("wq2", 0)
                wk2 = dma_w8slice("wk2", DIM)
                dma_xT8(1024, 2048)

                # ---- pair-0 prologue ----
                q1p = alloc_q("q1p")
                q2p = alloc_q("q2p")
                emit_q_fill(q1p, wq1, 0, 256)
                emit_q_fill(q1p, wq1, 256, 512)
                k1T = alloc_k("k1T")
                k2T = alloc_k("k2T")
                v1p = alloc_v1()
                v18p = alloc_v18()
                pu = make_pair_units(wq1, wk1, wq2, wk2, wv1,
                                     q1p, q2p, k1T, k2T, v1p, v18p, xtb=True)
                pu.q1()  # already filled above; mark consumed
                pu.k1[0]()

                # ---- pair loop ----
                for p in range(NPAIR):
                    nx = p + 1
                    if nx < NPAIR:
                        wq1n = dma_wslice("wq1", w1_d, nx * 128)
                        wk1n = dma_wslice("wk1", w1_d, DIM + nx * 128)
                        wq2n = dma_w8slice("wq2", nx * 128)
                        wk2n = dma_w8slice("wk2", DIM + nx * 128)
                        wv1n = dma_wslice("wv1", wv1_d, nx * 128, bf16)
                        if nx == 4:
                            nc.sync.dma_start(
                                wpj[:],
                                wp_d[:, :].rearrange("(c p) n -> p c n", p=128),
                            )
                        q1pn = alloc_q("q1p")
                        q2pn = alloc_q("q2p")
                        k1Tn = alloc_k("k1T")
                        k2Tn = alloc_k("k2T")
                        v1pn = alloc_v1()
                        v18pn = alloc_v18()
                        pun = make_pair_units(wq1n, wk1n, wq2n, wk2n, wv1n,
                                              q1pn, q2pn, k1Tn, k2Tn,
                                              v1pn, v18pn)
                        global_q.append(pun.q1)
                        for u in pun.k1:
                            global_q.append(u)
                        for u in pun.v1:
                            global_q.append(u)
                        global_q.append(pun.q2)
                        for u in pun.k2:
                            global_q.append(u)
                        if p >= 1:
                            defer_q.append(
                                unit(lambda cc=p - 1: emit_chunk_finish(cc))
                            )
                    elif p == NPAIR - 1:
                        defer_q.append(unit(lambda: emit_chunk_finish(4)))
                        for j in range(QT):
                            for half in range(2):
                                defer_q.append(
                                    unit(
                                        lambda jj=j, hh=half: emit_proj_stage1(
                                            jj, hh
                                        )
                                    )
                                )

                    av1 = attn_chain(0, k1T, q1p, v1p,
                                     qk_hook=c1_qk_hook(pu),
                                     av_hook=c1_av_hook(pu))
                    av2 = attn_chain(0, k2T, q2p, v18p,
                                     qk_hook=c2_qk_hook(pu),
                                     av_hook=paced_av, fp8av=True)
                    combine(2 * p, av1, av2)
                    if p == NPAIR - 1:
                        defer_q.append(unit(lambda: emit_head_finish(10)))
                    av1 = attn_chain(1, k1T, q1p, v1p,
                                     qk_hook=paced_qk, av_hook=paced_av)
                    av2 = attn_chain(1, k2T, q2p, v18p,
                                     qk_hook=paced_qk, av_hook=paced_av,
                                     fp8av=True)
                    combine(2 * p + 1, av1, av2)
                    if p == NPAIR - 1:
                        emit_head_finish(11)

                    if nx < NPAIR:
                        k1T, k2T, v1p, v18p = k1Tn, k2Tn, v1pn, v18pn
                        q1p, q2p = q1pn, q2pn
                        pu = pun

                drain_all()

            # ---- tail: per query tile: chunk-5 finish -> rms -> proj ----
            with tc.tile_pool(name="proj", bufs=1) as prj:
                mv = prj.tile([128, QT, 2], f32, tag="mv")
                rms = prj.tile([128, QT], f32, tag="rms")
                eps_t = prj.tile([128, 1], f32, tag="eps_t")
                nc.vector.memset(eps_t[:], EPS)

                def emit_rms_j(j):
                    nc.vector.bn_aggr(out=mv[:, j, :], in_=stats[:, j])

                    nc.vector.tensor_tensor(
                        out=mv[:, j, 0:1],
                        in0=mv[:, j, 0:1],
                        in1=mv[:, j, 0:1],
                        op=mybir.AluOpType.mult,
                    )
                    nc.vector.tensor_tensor(
                        out=mv[:, j, 1:2],
                        in0=mv[:, j, 1:2],
                        in1=mv[:, j, 0:1],
                        op=mybir.AluOpType.add,
                    )
                    nc.scalar.activation(
                        rms[:, j : j + 1],
                        mv[:, j, 1:2],
                        mybir.ActivationFunctionType.Sqrt,
                        bias=eps_t[:],
                        scale=1.0,
                    )
                    nc.vector.reciprocal(rms[:, j : j + 1], rms[:, j : j + 1])

                def emit_proj_j(j):
                    # stage 2: chunk-5 contribution joins the stage-1
                    # accumulator; rms folds into the epilogue:
                    # (y*rms) @ Wp = (y @ Wp) * rms  (rms is per-token)
                    jr = slice(j * 128, (j + 1) * 128)
                    osb2 = prj.tile(
                        [128, DIM], f32, tag="out_sb", bufs=2, name="osb2"
                    )
                    osb3 = prj.tile(
                        [128, DIM], f32, tag="out_sb3", bufs=2, name="osb3"
                    )
                    for half in range(2):
                        ps = psp.tile([128, 512], f32, tag="mm", bufs=2, name="psp2")
                        nc.tensor.matmul(
                            ps[:, 0:384],
                            yT[:, 5, jr],
                            wpj[:, 5, half * 384 : (half + 1) * 384],
                            start=True,
                            stop=True,
                        )
                        hs = slice(half * 384, (half + 1) * 384)
                        nc.vector.tensor_tensor(
                            out=osb2[:, hs],
                            in0=pacc[:, j, hs],
                            in1=ps[:, 0:384],
                            op=mybir.AluOpType.add,
                        )
                        nc.scalar.activation(
                            osb3[:, hs],
                            osb2[:, hs],
                            mybir.ActivationFunctionType.Copy,
                            scale=rms[:, j : j + 1],
                        )
                        nc.sync.dma_start(
                            out_d[j * 128 : (j + 1) * 128, hs], osb3[:, hs]
                        )

                for j in range(QT):
                    emit_rms_j(j)
                    emit_proj_j(j)

    _split_waits(nc)
    return nc


def kernel(x, W_qkv1, W_qkv2, W_proj, b_proj, norm_w, lambda_1, lambda_2, xpos):
    import ml_dtypes
    from concourse.bass_utils import run_bass_kernel_spmd

    if "nc" not in _cache:
        _cache["nc"] = _build()
    nc = _cache["nc"]

    bf16 = ml_dtypes.bfloat16
    e4m3 = ml_dtypes.float8_e4m3
    x = np.asarray(x, dtype=np.float32)
    w1 = np.asarray(W_qkv1, dtype=np.float32)
    w2 = np.asarray(W_qkv2, dtype=np.float32)
    w1qk = np.ascontiguousarray(w1[:, : 2 * DIM])
    w28 = np.clip(w2[:, : 2 * DIM], -240, 240).astype(e4m3)
    wv1b = np.ascontiguousarray(w1[:, 2 * DIM :]).astype(bf16)
    wpb = np.ascontiguousarray(
        np.asarray(norm_w, dtype=np.float32)[:, None]
        * np.asarray(W_proj, dtype=np.float32)
    ).astype(bf16)
    bp = np.ascontiguousarray(np.asarray(b_proj, dtype=np.float32))
    lam = np.ascontiguousarray(
        (
            np.asarray(lambda_1, dtype=np.float32)
            - np.asarray(lambda_2, dtype=np.float32)
            + LAMBDA_INIT
        ).astype(np.float32)
    )

    in_maps = []
    for c in range(NCORES):
        b, qi = c // 4, c % 4
        # rotate tokens so this core's query block comes first; attention
        # is permutation-invariant over keys so only q/out order matters
        xr = np.roll(x[b], -qi * NQ, axis=0)
        xrT = np.ascontiguousarray(xr.T)
        in_maps.append(
            {
                "xTp": xrT,
                "x8": np.clip(xrT, -240, 240).astype(e4m3),
                "w1": w1qk,
                "w28": w28,
                "wv1b": wv1b,
                "wp": wpb,
                "bp": bp,
                "lam": lam,
            }
        )

    res = run_bass_kernel_spmd(nc, in_maps, core_ids=list(range(NCORES)))
    out = np.empty((B, N, DIM), dtype=np.float32)
    for c in range(NCORES):
        b, qi = c // 4, c % 4
        out[b, qi * NQ : (qi + 1) * NQ, :] = res.results[c]["out"]
    return out



# revision 52
# speedup vs baseline: 1.1877x; 1.0993x over previous
"""DIFF-Attention Trainium2 kernel (v4: fp8 DoubleRow attn2 path).

Problem: B=2, N=2048, DIM=768, H=12, HD=64, two qkv projections, two
softmax attention maps, diff = attn1 - lam*attn2, out = diff @ v1,
RMSNorm, proj.

v4: the attn2 branch enters the output scaled by lam = l1-l2+0.1 ~=
0.108, so its quantization noise is suppressed ~9x. Exploit with
fp8e4m3 + MatmulPerfMode.DoubleRow (2 contraction rows packed per
matmul at 0.5 cycles/row) on: the k2/q2 qkv GEMMs (host-cast x8/w28
inputs), and AV2 (E2 = exp(S2*0.125 - ESH) written fp8 by the
activation, v18 = fp8 copy of v1). ESH shifts exp output under
e4m3's +-240 range (softmax is shift-invariant). QK2 stays bf16:
feeding it fp8 would need a [32,2,free] relayout that costs more
DVE than the PE it saves.

Sharding: 8 cores; core c handles batch b = c//4 and query tokens
[512*(c%4), 512*(c%4)+512). Attention is permutation-invariant over
keys, so the host hands each core x^T with the token axis ROTATED so
that the core's 512 query tokens come first: q GEMMs read the first
512 columns of the same xT tile the k/v GEMMs read (no separate xq
DMA). Each core computes k1/k2/v1 for the whole (permuted) batch and
q/attention/norm/proj only for its 512 query tokens. No collectives.

Structure:
  - Each attention chain runs its 8 QK+exp groups first (8 E tiles
    buffered), then 4 query-tile AV passes with out[query, hd]
    (queries on psum partitions, 65-wide moving dim). Sequential
    per-region psum accumulation — interleaved start/stop groups
    within one psum bank silently corrupt results.
  - combine() reads the AV psums directly (no PE transpose of O).
  - v1 is computed per head-pair (prefetched like k/q) as a bf16
    token-major GEMM from an on-chip bf16 copy of x^T.
  - Work units (next-pair k/q/v GEMMs, prev-chunk Y transpose+stats)
    are popped one per attention group. Units only needed by LATER
    chains (q2/k2 fills, chunk finish) are carryable across the pair
    boundary so the last pairs keep PE fed.
  - Tail pipelines per query tile: Y-chunk-5 transpose -> rms -> proj
    -> epilogue -> half-row output DMAs.

Numerics: f32r (tf32-like) qkv/QK GEMMs, bf16 exp(S) / AV / v1 / proj,
fp32 PSUM accumulation, RMSNorm stats in fp32.
"""

import os
import numpy as np

B, N, DIM, H, HD = 2, 2048, 768, 12, 64
NQ = 512            # query tokens per core
LAMBDA_INIT = 0.1
EPS = 1e-6
NCORES = 8
ESH = 5.0           # exp shift for fp8 E2 (max observed logit ~8.7; e4m3 max 240)
_LOG2E = 1.4426950408889634
SCH_A = float(0.125 * _LOG2E * (1 << 23))
SCH_B = float((1 << 23) * (127.0 - ESH * _LOG2E) - 366393.0)

_cache = {}


def _split_waits(nc, max_waits=1):
    """The walrus build in this environment rejects instructions carrying
    more than one explicit sync wait. Hoist excess waits onto NoOps
    inserted just before, on the same engine (same-engine program order
    makes this semantically equivalent)."""
    import concourse.mybir as mybir

    ctr = 0
    for f in nc.m.functions:
        for b in f.blocks:
            out = []
            changed = False
            for inst in b.instructions:
                si = inst.sync_info
                waits = list(si.on_wait) if si is not None and si.on_wait else []
                if len(waits) > max_waits:
                    changed = True
                    keep = waits[-max_waits:]
                    excess = waits[:-max_waits]
                    for i in range(0, len(excess), max_waits):
                        ctr += 1
                        nop = mybir.InstNoOp(
                            name=f"I-waitsplit-{ctr}", ins=[], outs=[]
                        )
                        nop.engine = inst.engine
                        nop.sync_info = mybir.SyncInfo(
                            on_wait=excess[i : i + max_waits], on_update=[]
                        )
                        out.append(nop)
                    inst.sync_info = mybir.SyncInfo(
                        on_wait=keep,
                        on_update=list(si.on_update) if si.on_update else [],
                    )
                out.append(inst)
            if changed:
                b.instructions = out


def _build():
    import concourse.bass as bass
    import concourse.mybir as mybir
    import concourse.tile as tile
    from concourse.masks import make_identity
    from collections import deque

    f32 = mybir.dt.float32
    f32r = mybir.dt.float32r
    bf16 = mybir.dt.bfloat16
    fp8 = mybir.dt.float8e4
    i32 = mybir.dt.int32
    DR = mybir.MatmulPerfMode.DoubleRow
    SCHG = int(os.environ.get("SCHG", "0"))

    nc = bass.Bass(trn_type="TRN2")

    xb_d = nc.dram_tensor("xb", [DIM, N], bf16, kind="ExternalInput")
    x8_d = nc.dram_tensor("x8", [DIM, N], fp8, kind="ExternalInput")
    w1_d = nc.dram_tensor("w1", [DIM, 2 * DIM], bf16, kind="ExternalInput")
    w28_d = nc.dram_tensor("w28", [DIM, 2 * DIM], fp8, kind="ExternalInput")
    wv1_d = nc.dram_tensor("wv1b", [DIM, DIM], bf16, kind="ExternalInput")
    wp_d = nc.dram_tensor("wp", [DIM, DIM], bf16, kind="ExternalInput")
    bp_d = nc.dram_tensor("bp", [DIM], f32, kind="ExternalInput")
    lam_d = nc.dram_tensor("lam", [H], f32, kind="ExternalInput")
    out_d = nc.dram_tensor("out", [NQ, DIM], f32, kind="ExternalOutput")
    # DRAM bounce scratch for the q2/k2 partition-fold relayout
    # (alternating by pair parity so same-queue FIFO orders reuse)
    k2b_d = [
        nc.dram_tensor(f"k2b{i}", [128, N], fp8, kind="Internal")
        for i in range(2)
    ]
    q2b_d = [
        nc.dram_tensor(f"q2b{i}", [128, NQ], fp8, kind="Internal")
        for i in range(2)
    ]

    C = 6          # 768 / 128 feature chunks
    NPAIR = 6      # head pairs
    TT = 16        # token tiles of 128 in N
    QT = 4         # query sub-tiles of 128 in NQ

    with tile.TileContext(nc) as tc:
        with (
            tc.tile_pool(name="persist", bufs=1) as pp,
            tc.tile_pool(name="psum", bufs=1, space="PSUM") as psp,
        ):
            # ---- constants / small tiles ----
            ident = pp.tile([128, 128], f32, tag="ident")
            make_identity(nc, ident[:])
            lam_b = pp.tile([128, H], f32, tag="lam_b")
            nc.gpsimd.dma_start(
                out=lam_b[:],
                in_=bass.AP(tensor=lam_d, offset=0, ap=[[0, 128], [1, H]]),
            )
            bp_row = pp.tile([1, DIM], f32, tag="bp_row")
            nc.gpsimd.dma_start(
                out=bp_row[:],
                in_=bass.AP(tensor=bp_d, offset=0, ap=[[0, 1], [1, DIM]]),
            )
            ones_col = pp.tile([1, 128], f32, tag="ones_col")
            nc.vector.memset(ones_col[:], 1.0)
            esh_t = pp.tile([128, 1], f32, tag="esh_t")
            nc.vector.memset(esh_t[:], -ESH)

            # ---- persistent big tiles ----
            xT8 = pp.tile([128, C, N], fp8, tag="xT8")
            xTb = pp.tile([128, C, N], bf16, tag="xTb")
            Y = pp.tile([128, QT, DIM], f32, tag="Y")
            yT = pp.tile([128, C, NQ], bf16, tag="yT")
            stats = pp.tile([128, QT, C + 1, 6], f32, tag="stats")
            wpj = pp.tile([128, C, DIM], bf16, tag="wpj")

            def dma_xTb(lo, hi):
                nc.sync.dma_start(
                    xTb[:, :, lo:hi],
                    xb_d[:, lo:hi].rearrange("(c p) m -> p c m", p=128),
                )

            def dma_xT8(lo, hi):
                nc.sync.dma_start(
                    xT8[:, :, lo:hi],
                    x8_d[:, lo:hi].rearrange("(c p) m -> p c m", p=128),
                )

            with (
                tc.tile_pool(name="pairs", bufs=2) as wpool,
                tc.tile_pool(name="epool", bufs=int(os.environ.get("EPBUFS", "17"))) as ep,
                tc.tile_pool(name="schpool", bufs=2) as schp,
            ):
                # ---- weight slice DMA + GEMM emit helpers ----
                def dma_wslice(tag, src_w, col0, dt=bf16):
                    t = wpool.tile([128, C, 128], dt, tag=tag, name=tag)
                    nc.sync.dma_start(
                        t[:],
                        src_w[:, col0 : col0 + 128].rearrange(
                            "(c p2) n -> p2 c n", p2=128
                        ),
                    )
                    return t

                def dma_w8slice(tag, col0):
                    # fp8 DoubleRow layout: [part, chunk-pair, row-half, col]
                    t = wpool.tile([128, 3, 2, 128], fp8, tag=tag, name=tag)
                    nc.sync.dma_start(
                        t[:],
                        w28_d[:, col0 : col0 + 128].rearrange(
                            "(cc two p2) n -> p2 cc two n", p2=128, two=2
                        ),
                    )
                    return t

                def alloc_q(tag):
                    return wpool.tile([128, NQ], bf16, tag=tag, name=tag)

                def emit_q_fill(qp, wq, lo=0, hi=NQ):
                    ps = psp.tile([128, 512], f32, tag="mm", bufs=2, name="psq")
                    w = hi - lo
                    for c in range(C):
                        nc.tensor.matmul(
                            ps[:, 0:w],
                            wq[:, c, :],
                            xTb[:, c, lo:hi],
                            start=(c == 0),
                            stop=(c == C - 1),
                        )
                    nc.vector.tensor_copy(qp[:, lo:hi], ps[:, 0:w])

                def alloc_k(tag):
                    return wpool.tile([128, N], bf16, tag=tag, name=tag)

                def emit_k_gemm(kt, wk, mt):
                    ps = psp.tile([128, 512], f32, tag="mm", bufs=2, name="psk")
                    for c in range(C):
                        nc.tensor.matmul(
                            ps[:],
                            wk[:, c, :],
                            xTb[:, c, mt * 512 : (mt + 1) * 512],
                            start=(c == 0),
                            stop=(c == C - 1),
                        )
                    nc.vector.tensor_copy(kt[:, mt * 512 : (mt + 1) * 512], ps[:])

                # fp8 q2/k2: GEMM -> fp8 staging tile -> DRAM bounce ->
                # [32, head, d-half, tok] fold for DoubleRow QK2
                def emit_q_fill8(q2f8, wq8, par):
                    ps = psp.tile([128, 512], f32, tag="mm", bufs=2, name="psq8")
                    for cc in range(3):
                        nc.tensor.matmul(
                            ps[:],
                            wq8[:, cc],
                            xT8[:, 2 * cc : 2 * cc + 2, 0:NQ],
                            start=(cc == 0),
                            stop=(cc == 2),
                            perf_mode=DR,
                        )
                    nc.vector.tensor_copy(q2f8[:], ps[:])
                    nc.sync.dma_start(q2b_d[par][:], q2f8[:])

                def emit_k_gemm8(k2f8, wk8, mt, par):
                    ps = psp.tile([128, 512], f32, tag="mm", bufs=2, name="psk8")
                    for cc in range(3):
                        nc.tensor.matmul(
                            ps[:],
                            wk8[:, cc],
                            xT8[:, 2 * cc : 2 * cc + 2, mt * 512 : (mt + 1) * 512],
                            start=(cc == 0),
                            stop=(cc == 2),
                            perf_mode=DR,
                        )
                    sl = slice(mt * 512, (mt + 1) * 512)
                    nc.vector.tensor_copy(k2f8[:, sl], ps[:])
                    nc.sync.dma_start(k2b_d[par][:, sl], k2f8[:, sl])

                def emit_k2_fold(k28t, par):
                    for h in range(2):
                        nc.sync.dma_start(
                            k28t[:, h],
                            k2b_d[par][64 * h : 64 * h + 64, :].rearrange(
                                "(two p) n -> p two n", p=32
                            ),
                        )

                def emit_q2_fold(q28t, par):
                    for h in range(2):
                        nc.sync.dma_start(
                            q28t[:, h],
                            q2b_d[par][64 * h : 64 * h + 64, :].rearrange(
                                "(two p) n -> p two n", p=32
                            ),
                        )

                def alloc_v1():
                    v = wpool.tile(
                        [128, TT, 2, HD + 1], bf16, tag="v1p", name="v1p"
                    )
                    nc.vector.memset(v[:, :, :, HD : HD + 1], 1.0)
                    return v

                def alloc_v18():
                    v = wpool.tile(
                        [128, TT, 2, HD + 1], fp8, tag="v18", name="v18"
                    )
                    nc.vector.memset(v[:, :, :, HD : HD + 1], 1.0)
                    return v

                def emit_v1_unit(vdst, v8dst, wv, u):
                    # token tiles 2u, 2u+1; out token-major [tok, 2*HD]
                    ps = psp.tile([128, 512], f32, tag="mm", bufs=2, name="psv")
                    for half in range(2):
                        t = 2 * u + half
                        for c in range(C):
                            nc.tensor.matmul(
                                ps[:, half * 128 : (half + 1) * 128],
                                xTb[:, c, t * 128 : (t + 1) * 128],
                                wv[:, c, :],
                                start=(c == 0),
                                stop=(c == C - 1),
                            )
                    nc.vector.tensor_copy(
                        vdst[:, 2 * u : 2 * u + 2, :, 0:HD],
                        ps[:, 0:256].rearrange("p (t h d) -> p t h d", t=2, h=2),
                    )
                    nc.vector.tensor_copy(
                        v8dst[:, 2 * u : 2 * u + 2, :, 0:HD],
                        ps[:, 0:256].rearrange("p (t h d) -> p t h d", t=2, h=2),
                    )

                # ---- proj stage 1: accumulate chunks 0-4 of the proj
                # GEMM into an SBUF accumulator (runs as late deferred
                # units inside pair 5); chunk 5 joins at the tail ----
                pacc = pp.tile([128, QT, DIM], f32, tag="pacc")

                def emit_proj_stage1(j, half):
                    # chunks 0-4 of y@Wp, plus bias via rank-1 ones x bp
                    jr = slice(j * 128, (j + 1) * 128)
                    hs = slice(half * 384, (half + 1) * 384)
                    ps = psp.tile([128, 512], f32, tag="mm", bufs=2, name="ps1")
                    nc.tensor.matmul(
                        ps[:, 0:384],
                        ones_col[:, 0:128],
                        bp_row[:, hs],
                        start=True,
                        stop=False,
                    )
                    for c in range(5):
                        nc.tensor.matmul(
                            ps[:, 0:384],
                            yT[:, c, jr],
                            wpj[:, c, hs],
                            start=False,
                            stop=(c == 4),
                        )
                    nc.vector.tensor_copy(pacc[:, j, hs], ps[:, 0:384])

                # ---- per-head finish for chunk 5 (heads 10, 11) ----
                def emit_head_finish_j(h, j):
                    hh = h % 2
                    tp = psp.tile([128, 512], f32, tag="mm", bufs=2, name="tp")
                    nc.tensor.transpose(
                        tp[0:64, 0:128],
                        Y[:, j, h * 64 : (h + 1) * 64],
                        ident[:],
                    )
                    nc.vector.tensor_copy(
                        yT[hh * 64 : (hh + 1) * 64, 5, j * 128 : (j + 1) * 128],
                        tp[0:64, 0:128],
                    )
                    nc.vector.bn_stats(
                        out=stats[:, j, 5 + hh, :],
                        in_=Y[:, j, h * 64 : (h + 1) * 64],
                    )

                def emit_head_finish(h):
                    for j in range(QT):
                        emit_head_finish_j(h, j)

                # ---- chunk finish: transpose Y chunk + bn stats ----
                def emit_chunk_finish_j(cc, j):
                    tp = psp.tile([128, 512], f32, tag="mm", bufs=2, name="tp")
                    nc.tensor.transpose(
                        tp[:, 0:128],
                        Y[:, j, cc * 128 : (cc + 1) * 128],
                        ident[:],
                    )
                    nc.vector.tensor_copy(
                        yT[:, cc, j * 128 : (j + 1) * 128], tp[:, 0:128]
                    )
                    nc.vector.bn_stats(
                        out=stats[:, j, cc, :],
                        in_=Y[:, j, cc * 128 : (cc + 1) * 128],
                    )

                def emit_chunk_finish(cc):
                    for j in range(QT):
                        emit_chunk_finish_j(cc, j)

                # ---- one-shot work units + paced global queue ----
                # Every prefetched GEMM is a one-shot closure. The pair
                # that OWNS a unit force-runs it at the latest safe point
                # (deadline hooks inside its own chains); paced pops from
                # the global queue run units early when PE has slack, so
                # leftover work naturally flows into the last pairs where
                # the exp stream would otherwise throttle PE.
                def unit(fn):
                    st = [False]

                    def run():
                        if st[0]:
                            return False
                        st[0] = True
                        fn()
                        return True

                    run.consume = lambda: st.__setitem__(0, True)
                    return run

                global_q = deque()
                defer_q = deque()

                def pop1(_=None):
                    while global_q:
                        if global_q.popleft()():
                            return
                    while defer_q:
                        if defer_q.popleft()():
                            return

                def drain_all():
                    while global_q:
                        global_q.popleft()()
                    while defer_q:
                        defer_q.popleft()()

                PACE = int(os.environ.get("PACE", "1"))

                class PU:
                    pass

                def make_pair_units(wq1s, wk1s, wq2s8, wk2s8, wv1s,
                                    q1t, k1t, v1t, v8t,
                                    q2f8t, k2f8t, q28t, k28t, par):
                    pu = PU()
                    pu.q1 = unit(lambda: emit_q_fill(q1t, wq1s))
                    pu.k1 = [
                        unit(lambda m=m: emit_k_gemm(k1t, wk1s, m))
                        for m in range(4)
                    ]
                    pu.v1 = [
                        unit(lambda u=u: emit_v1_unit(v1t, v8t, wv1s, u))
                        for u in range(8)
                    ]
                    pu.q2 = unit(lambda: emit_q_fill8(q2f8t, wq2s8, par))
                    pu.k2 = [
                        unit(lambda m=m: emit_k_gemm8(k2f8t, wk2s8, m, par))
                        for m in range(4)
                    ]
                    pu.k2f = unit(lambda: emit_k2_fold(k28t, par))
                    pu.q2f = unit(lambda: emit_q2_fold(q28t, par))
                    return pu

                def c1_qk_hook(pu, jit=False):
                    def h(g):
                        pu.q1()
                        for m in range(g // 2 + 1):
                            pu.k1[m]()
                        if jit:
                            # pair 0: wv1/x8/w28 DMAs land after xTb; forcing
                            # v1/k2 here would stall the in-order PE on DMA
                            # waits while QK work is ready. Late-force v1
                            # only; k2/q2/folds run at c2's g0 safety net.
                            if g >= 6:
                                pu.v1[g - 6]()
                            return
                        if g >= 2:
                            pu.v1[g - 2]()
                        # fp8 k2/q2 pipeline: GEMMs + bounce-out by g==7,
                        # fold-in DMAs issued right after (chain2 needs the
                        # folded tiles before its first QK matmul)
                        if g == 1:
                            pu.q2()
                        if g % 2 == 1:
                            pu.k2[(g - 1) // 2]()
                        if g == 7:
                            pu.k2f()
                            pu.q2f()
                        if qk_pop_ok(g):
                            pop1()

                    return h

                def c1_av_hook(pu):
                    def h(j):
                        if j == 0:
                            for u in pu.v1:
                                u()
                        else:
                            pop1()

                    return h

                def c2_qk_hook(pu):
                    def h(g):
                        if g == 0:
                            # safety: idempotent if already forced in chain1
                            pu.q2()
                            for u in pu.k2:
                                u()
                            pu.k2f()
                            pu.q2f()
                        if g % PACE == 1:
                            pop1()

                    return h

                QKPOP = os.environ.get("QKPOP", "none")

                def qk_pop_ok(g):
                    if QKPOP == "none":
                        return False
                    if QKPOP == "g7":
                        return g == 7
                    if QKPOP == "odd":
                        return g % 2 == 1
                    return True

                DEFPOP = int(os.environ.get("DEFPOP", "1"))
                # guard: during the LAST pair's QK phases the global queue
                # is empty and every pop would stuff a ~1us deferred unit
                # (proj stage1 etc.) between QK groups, stalling the
                # ACT-critical exp stream; hold them for the AV phases
                qk_nodefer = [False]

                def paced_qk(g):
                    if qk_pop_ok(g):
                        pop1()
                    elif DEFPOP and defer_q and not global_q \
                            and not qk_nodefer[0]:
                        pop1()

                AVJ = int(os.environ.get("AVJ", "1"))

                def paced_av(j):
                    if j >= AVJ:
                        pop1()

                # ---- attention chain, split into QK and AV phases so
                # chain2's QK (and its exp stream) overlaps chain1's AV ----
                def qk_phase(hh, kt, qp, qk_hook=None,
                             fp8av=False, k8q8=None):
                    po = hh * HD
                    ets = []
                    for g in range(8):
                        if qk_hook is not None:
                            qk_hook(g)
                        qk = psp.tile(
                            [128, 2, 512], f32, tag="qk", bufs=2, name="qk"
                        )
                        for g2 in range(2):
                            mc = g * 2 + g2
                            if k8q8 is not None:
                                k28t, q28t = k8q8
                                nc.tensor.matmul(
                                    qk[:, g2, :],
                                    k28t[:, hh, :, mc * 128 : (mc + 1) * 128],
                                    q28t[:, hh],
                                    start=True,
                                    stop=True,
                                    perf_mode=DR,
                                )
                            else:
                                nc.tensor.matmul(
                                    qk[:, g2, :],
                                    kt[po : po + 64, mc * 128 : (mc + 1) * 128],
                                    qp[po : po + 64, :],
                                    start=True,
                                    stop=True,
                                )
                        if fp8av and g < SCHG:
                            # Schraudolph exp2 on DVE + fp8 cast on Pool:
                            # bits(2^y) ~= int(y*2^23 + magic); precision
                            # ~3% — absorbed by the lam~0.108 suppression
                            t32 = schp.tile(
                                [128, 2, 512], i32, tag="sch", name="sch"
                            )
                            nc.vector.tensor_scalar(
                                out=t32[:],
                                in0=qk[:],
                                scalar1=SCH_A,
                                scalar2=SCH_B,
                                op0=mybir.AluOpType.mult,
                                op1=mybir.AluOpType.add,
                            )
                            e_t = ep.tile(
                                [128, 2, 512], fp8, tag="E8", name="e8_t"
                            )
                            nc.gpsimd.tensor_copy(e_t[:], t32[:].bitcast(f32))
                        elif fp8av:
                            e_t = ep.tile(
                                [128, 2, 512], fp8, tag="E8", name="e8_t"
                            )
                            nc.scalar.activation(
                                e_t[:],
                                qk[:],
                                mybir.ActivationFunctionType.Exp,
                                bias=esh_t[:],
                                scale=0.125,
                            )
                        else:
                            e_t = ep.tile(
                                [128, 2, 512], bf16, tag="E", name="e_t"
                            )
                            nc.scalar.activation(
                                e_t[:],
                                qk[:],
                                mybir.ActivationFunctionType.Exp,
                                scale=0.125,
                            )
                        ets.append(e_t)
                    return ets

                def av_phase(hh, v1p, ets, av_hook=None, fp8av=False):
                    av = psp.tile(
                        [128, QT, HD + 1], f32, tag="av", bufs=2, name="av"
                    )
                    for j in range(QT):
                        if av_hook is not None:
                            av_hook(j)
                        if fp8av:
                            for g in range(8):
                                nc.tensor.matmul(
                                    av[:, j, :],
                                    ets[g][:, :, j * 128 : (j + 1) * 128],
                                    v1p[:, 2 * g : 2 * g + 2, hh, :],
                                    start=(g == 0),
                                    stop=(g == 7),
                                    perf_mode=DR,
                                )
                        else:
                            for mc in range(16):
                                nc.tensor.matmul(
                                    av[:, j, :],
                                    ets[mc // 2][:, mc % 2, j * 128 : (j + 1) * 128],
                                    v1p[:, mc, hh, :],
                                    start=(mc == 0),
                                    stop=(mc == 15),
                                )
                    return av

                def combine(h, av1, av2):
                    # reads the AV psums directly; av1 released first so
                    # the next chain can reuse the psum buffer sooner
                    r1 = wpool.tile([128, QT, 1], f32, tag="r1", bufs=2, name="r1")
                    nc.vector.reciprocal(r1[:], av1[:, :, HD : HD + 1])
                    t1 = wpool.tile([128, QT, HD], f32, tag="t1", bufs=1, name="t1")
                    r1b = bass.AP(
                        tensor=r1.tensor,
                        offset=r1.offset,
                        ap=[r1.ap[0], r1.ap[1], [0, HD]],
                    )
                    nc.vector.tensor_tensor(
                        out=t1[:],
                        in0=av1[:, :, 0:HD],
                        in1=r1b,
                        op=mybir.AluOpType.mult,
                    )
                    r2 = wpool.tile([128, QT, 1], f32, tag="r2", bufs=2, name="r2")
                    nc.vector.reciprocal(r2[:], av2[:, :, HD : HD + 1])
                    lam_h = lam_b[:, h : h + 1]
                    lam_bc = bass.AP(
                        tensor=lam_h.tensor,
                        offset=lam_h.offset,
                        ap=[lam_h.ap[0], [0, QT], [0, 1]],
                    )
                    nc.vector.tensor_tensor(
                        out=r2[:], in0=r2[:], in1=lam_bc, op=mybir.AluOpType.mult
                    )
                    t2 = wpool.tile([128, QT, HD], f32, tag="t2", bufs=1, name="t2")
                    r2b = bass.AP(
                        tensor=r2.tensor,
                        offset=r2.offset,
                        ap=[r2.ap[0], r2.ap[1], [0, HD]],
                    )
                    nc.vector.tensor_tensor(
                        out=t2[:],
                        in0=av2[:, :, 0:HD],
                        in1=r2b,
                        op=mybir.AluOpType.mult,
                    )
                    nc.vector.tensor_tensor(
                        out=Y[:, :, h * 64 : (h + 1) * 64],
                        in0=t1[:],
                        in1=t2[:],
                        op=mybir.AluOpType.subtract,
                    )

                # ---- startup DMAs (issue order = DMA device order) ----
                # chain1 QK of pair 0 paces the whole ramp: its xTb/w1
                # stream goes first; wv1 (needed at c1AV) and the fp8 side
                # (needed at c2QK, ~8 exp-groups later) follow
                dma_xTb(0, 256)
                wq1 = dma_wslice("wq1", w1_d, 0)
                dma_xTb(256, 512)
                wk1 = dma_wslice("wk1", w1_d, DIM)
                dma_xTb(512, 1024)
                dma_xTb(1024, 1536)
                dma_xTb(1536, 2048)
                wv1 = dma_wslice("wv1", wv1_d, 0, bf16)
                dma_xT8(0, 1024)
                wq2 = dma_w8slice("wq2", 0)
                wk2 = dma_w8slice("wk2", DIM)
                dma_xT8(1024, 2048)

                def alloc_pair_fp8():
                    q2f8t = wpool.tile([128, NQ], fp8, tag="q2f8", name="q2f8")
                    k2f8t = wpool.tile([128, N], fp8, tag="k2f8", name="k2f8")
                    q28t = wpool.tile([32, 2, 2, NQ], fp8, tag="q28", name="q28")
                    k28t = wpool.tile([32, 2, 2, N], fp8, tag="k28", name="k28")
                    return q2f8t, k2f8t, q28t, k28t

                # ---- pair-0 prologue ----
                q1p = alloc_q("q1p")
                emit_q_fill(q1p, wq1, 0, 256)
                emit_q_fill(q1p, wq1, 256, 512)
                k1T = alloc_k("k1T")
                v1p = alloc_v1()
                v18p = alloc_v18()
                q2f8, k2f8, q28, k28 = alloc_pair_fp8()
                pu = make_pair_units(wq1, wk1, wq2, wk2, wv1,
                                     q1p, k1T, v1p, v18p,
                                     q2f8, k2f8, q28, k28, par=0)
                pu.q1.consume()  # filled above in halves
                pu.k1[0]()

                # ---- pair loop ----
                for p in range(NPAIR):
                    nx = p + 1
                    if nx < NPAIR:
                        wq1n = dma_wslice("wq1", w1_d, nx * 128)
                        wk1n = dma_wslice("wk1", w1_d, DIM + nx * 128)
                        wq2n = dma_w8slice("wq2", nx * 128)
                        wk2n = dma_w8slice("wk2", DIM + nx * 128)
                        wv1n = dma_wslice("wv1", wv1_d, nx * 128, bf16)
                        if nx == 4:
                            nc.sync.dma_start(
                                wpj[:],
                                wp_d[:, :].rearrange("(c p) n -> p c n", p=128),
                            )
                        q1pn = alloc_q("q1p")
                        k1Tn = alloc_k("k1T")
                        v1pn = alloc_v1()
                        v18pn = alloc_v18()
                        q2f8n, k2f8n, q28n, k28n = alloc_pair_fp8()
                        pun = make_pair_units(wq1n, wk1n, wq2n, wk2n, wv1n,
                                              q1pn, k1Tn, v1pn, v18pn,
                                              q2f8n, k2f8n, q28n, k28n,
                                              par=nx % 2)
                        global_q.append(pun.q1)
                        for u in pun.k1:
                            global_q.append(u)
                        for u in pun.v1:
                            global_q.append(u)
                        global_q.append(pun.q2)
                        for u in pun.k2:
                            global_q.append(u)
                        global_q.append(pun.k2f)
                        global_q.append(pun.q2f)
                        if p >= 1:
                            defer_q.append(
                                unit(lambda cc=p - 1: emit_chunk_finish(cc))
                            )
                    elif p == NPAIR - 1:
                        defer_q.append(unit(lambda: emit_chunk_finish(4)))
                        for j in range(QT):
                            for half in range(2):
                                defer_q.append(
                                    unit(
                                        lambda jj=j, hh=half: emit_proj_stage1(
                                            jj, hh
                                        )
                                    )
                                )

                    # all four QK phases first: 16 bf16 exps of run-ahead
                    # cover the fp8 side's GEMM+bounce+fold latency, and
                    # the exp stream never starves at phase boundaries
                    ets1a = qk_phase(0, k1T, q1p,
                                     qk_hook=c1_qk_hook(pu, jit=(p == 0)))
                    ets1b = qk_phase(1, k1T, q1p, qk_hook=paced_qk)
                    ets2a = qk_phase(0, None, None, qk_hook=c2_qk_hook(pu),
                                     fp8av=True, k8q8=(k28, q28))
                    ets2b = qk_phase(1, None, None, qk_hook=paced_qk,
                                     fp8av=True, k8q8=(k28, q28))
                    av1 = av_phase(0, v1p, ets1a, av_hook=c1_av_hook(pu))
                    av2 = av_phase(0, v18p, ets2a, av_hook=paced_av,
                                   fp8av=True)
                    combine(2 * p, av1, av2)
                    if p == NPAIR - 1:
                        defer_q.append(unit(lambda: emit_head_finish(10)))
                    av1 = av_phase(1, v1p, ets1b, av_hook=paced_av)
                    av2 = av_phase(1, v18p, ets2b, av_hook=paced_av,
                                   fp8av=True)
                    combine(2 * p + 1, av1, av2)

                    if nx < NPAIR:
                        k1T, v1p, v18p = k1Tn, v1pn, v18pn
                        q1p = q1pn
                        q28, k28 = q28n, k28n
                        pu = pun

                drain_all()

            # ---- tail: per query tile: chunk-5 finish -> rms -> proj ----
            with tc.tile_pool(name="proj", bufs=1) as prj:
                mv = prj.tile([128, QT, 2], f32, tag="mv")
                rms = prj.tile([128, QT], f32, tag="rms")
                eps_t = prj.tile([128, 1], f32, tag="eps_t")
                nc.vector.memset(eps_t[:], EPS)

                def emit_rms_j(j):
                    nc.vector.bn_aggr(out=mv[:, j, :], in_=stats[:, j])

                    nc.vector.tensor_tensor(
                        out=mv[:, j, 0:1],
                        in0=mv[:, j, 0:1],
                        in1=mv[:, j, 0:1],
                        op=mybir.AluOpType.mult,
                    )
                    nc.vector.tensor_tensor(
                        out=mv[:, j, 1:2],
                        in0=mv[:, j, 1:2],
                        in1=mv[:, j, 0:1],
                        op=mybir.AluOpType.add,
                    )
                    nc.scalar.activation(
                        rms[:, j : j + 1],
                        mv[:, j, 1:2],
                        mybir.ActivationFunctionType.Sqrt,
                        bias=eps_t[:],
                        scale=1.0,
                    )
                    nc.vector.reciprocal(rms[:, j : j + 1], rms[:, j : j + 1])

                def emit_proj_j(j):
                    # stage 2: chunk-5 contribution joins the stage-1
                    # accumulator; rms folds into the epilogue:
                    # (y*rms) @ Wp = (y @ Wp) * rms  (rms is per-token)
                    jr = slice(j * 128, (j + 1) * 128)
                    osb2 = prj.tile(
                        [128, DIM], f32, tag="out_sb", bufs=2, name="osb2"
                    )
                    osb3 = prj.tile(
                        [128, DIM], f32, tag="out_sb3", bufs=2, name="osb3"
                    )
                    for half in range(2):
                        ps = psp.tile([128, 512], f32, tag="mm", bufs=2, name="psp2")
                        nc.tensor.matmul(
                            ps[:, 0:384],
                            yT[:, 5, jr],
                            wpj[:, 5, half * 384 : (half + 1) * 384],
                            start=True,
                            stop=True,
                        )
                        hs = slice(half * 384, (half + 1) * 384)
                        nc.vector.tensor_tensor(
                            out=osb2[:, hs],
                            in0=pacc[:, j, hs],
                            in1=ps[:, 0:384],
                            op=mybir.AluOpType.add,
                        )
                        # rms scale on DVE (broadcast along free dim) so the
                        # tail doesn't queue behind the last exp instructions
                        rms_bc = bass.AP(
                            tensor=rms.tensor,
                            offset=rms.offset + j,
                            ap=[rms.ap[0], [0, 384]],
                        )
                        nc.vector.tensor_tensor(
                            out=osb3[:, hs],
                            in0=osb2[:, hs],
                            in1=rms_bc,
                            op=mybir.AluOpType.mult,
                        )
                        nc.sync.dma_start(
                            out_d[j * 128 : (j + 1) * 128, hs], osb3[:, hs]
                        )

                for j in range(QT):
                    emit_head_finish_j(11, j)
                    emit_rms_j(j)
                    emit_proj_j(j)

    _split_waits(nc)
    return nc


def kernel(x, W_qkv1, W_qkv2, W_proj, b_proj, norm_w, lambda_1, lambda_2, xpos):
    import ml_dtypes
    from concourse.bass_utils import run_bass_kernel_spmd

    if "nc" not in _cache:
        _cache["nc"] = _build()
    nc = _cache["nc"]

    bf16 = ml_dtypes.bfloat16
    e4m3 = ml_dtypes.float8_e4m3
    x = np.asarray(x, dtype=np.float32)
    w1 = np.asarray(W_qkv1, dtype=np.float32)
    w2 = np.asarray(W_qkv2, dtype=np.float32)
    w1qk = np.ascontiguousarray(w1[:, : 2 * DIM]).astype(bf16)
    w28 = np.clip(w2[:, : 2 * DIM], -240, 240).astype(e4m3)
    wv1b = np.ascontiguousarray(w1[:, 2 * DIM :]).astype(bf16)
    wpb = np.ascontiguousarray(
        np.asarray(norm_w, dtype=np.float32)[:, None]
        * np.asarray(W_proj, dtype=np.float32)
    ).astype(bf16)
    bp = np.ascontiguousarray(np.asarray(b_proj, dtype=np.float32))
    lam = np.ascontiguousarray(
        (
            np.asarray(lambda_1, dtype=np.float32)
            - np.asarray(lambda_2, dtype=np.float32)
            + LAMBDA_INIT
        ).astype(np.float32)
    )

    in_maps = []
    for c in range(NCORES):
        b, qi = c // 4, c % 4
        # rotate tokens so this core's query block comes first; attention
        # is permutation-invariant over keys so only q/out order matters
        xr = np.roll(x[b], -qi * NQ, axis=0)
        xrT = np.ascontiguousarray(xr.T)
        in_maps.append(
            {
                "xb": xrT.astype(bf16),
                "x8": np.clip(xrT, -240, 240).astype(e4m3),
                "w1": w1qk,
                "w28": w28,
                "wv1b": wv1b,
                "wp": wpb,
                "bp": bp,
                "lam": lam,
            }
        )

    res = run_bass_kernel_spmd(nc, in_maps, core_ids=list(range(NCORES)))
    out = np.empty((B, N, DIM), dtype=np.float32)
    for c in range(NCORES):
        b, qi = c // 4, c % 4
        out[b, qi * NQ : (qi + 1) * NQ, :] = res.results[c]["out"]
    return out



# revision 72
# speedup vs baseline: 1.1998x; 1.0102x over previous
"""DIFF-Attention Trainium2 kernel (v4: fp8 DoubleRow attn2 path).

Problem: B=2, N=2048, DIM=768, H=12, HD=64, two qkv projections, two
softmax attention maps, diff = attn1 - lam*attn2, out = diff @ v1,
RMSNorm, proj.

v4: the attn2 branch enters the output scaled by lam = l1-l2+0.1 ~=
0.108, so its quantization noise is suppressed ~9x. Exploit with
fp8e4m3 + MatmulPerfMode.DoubleRow (2 contraction rows packed per
matmul at 0.5 cycles/row) on: the k2/q2 qkv GEMMs (host-cast x8/w28
inputs), and AV2 (E2 = exp(S2*0.125 - ESH) written fp8 by the
activation, v18 = fp8 copy of v1). ESH shifts exp output under
e4m3's +-240 range (softmax is shift-invariant). QK2 stays bf16:
feeding it fp8 would need a [32,2,free] relayout that costs more
DVE than the PE it saves.

Sharding: 8 cores; core c handles batch b = c//4 and query tokens
[512*(c%4), 512*(c%4)+512). Attention is permutation-invariant over
keys, so the host hands each core x^T with the token axis ROTATED so
that the core's 512 query tokens come first: q GEMMs read the first
512 columns of the same xT tile the k/v GEMMs read (no separate xq
DMA). Each core computes k1/k2/v1 for the whole (permuted) batch and
q/attention/norm/proj only for its 512 query tokens. No collectives.

Structure:
  - Each attention chain runs its 8 QK+exp groups first (8 E tiles
    buffered), then 4 query-tile AV passes with out[query, hd]
    (queries on psum partitions, 65-wide moving dim). Sequential
    per-region psum accumulation — interleaved start/stop groups
    within one psum bank silently corrupt results.
  - combine() reads the AV psums directly (no PE transpose of O).
  - v1 is computed per head-pair (prefetched like k/q) as a bf16
    token-major GEMM from an on-chip bf16 copy of x^T.
  - Work units (next-pair k/q/v GEMMs, prev-chunk Y transpose+stats)
    are popped one per attention group. Units only needed by LATER
    chains (q2/k2 fills, chunk finish) are carryable across the pair
    boundary so the last pairs keep PE fed.
  - Tail pipelines per query tile: Y-chunk-5 transpose -> rms -> proj
    -> epilogue -> half-row output DMAs.

Numerics: f32r (tf32-like) qkv/QK GEMMs, bf16 exp(S) / AV / v1 / proj,
fp32 PSUM accumulation, RMSNorm stats in fp32.
"""

import os
import numpy as np

B, N, DIM, H, HD = 2, 2048, 768, 12, 64
NQ = 512            # query tokens per core
LAMBDA_INIT = 0.1
EPS = 1e-6
NCORES = 8
ESH = 5.0           # exp shift for fp8 E2 (max observed logit ~8.7; e4m3 max 240)
_LOG2E = 1.4426950408889634
SCH_A = float(0.125 * _LOG2E * (1 << 23))
SCH_B = float((1 << 23) * (127.0 - ESH * _LOG2E) - 366393.0)

_cache = {}


def _split_waits(nc, max_waits=1):
    """The walrus build in this environment rejects instructions carrying
    more than one explicit sync wait. Hoist excess waits onto NoOps
    inserted just before, on the same engine (same-engine program order
    makes this semantically equivalent)."""
    import concourse.mybir as mybir

    ctr = 0
    for f in nc.m.functions:
        for b in f.blocks:
            out = []
            changed = False
            for inst in b.instructions:
                si = inst.sync_info
                waits = list(si.on_wait) if si is not None and si.on_wait else []
                if len(waits) > max_waits:
                    changed = True
                    keep = waits[-max_waits:]
                    excess = waits[:-max_waits]
                    for i in range(0, len(excess), max_waits):
                        ctr += 1
                        nop = mybir.InstNoOp(
                            name=f"I-waitsplit-{ctr}", ins=[], outs=[]
                        )
                        nop.engine = inst.engine
                        nop.sync_info = mybir.SyncInfo(
                            on_wait=excess[i : i + max_waits], on_update=[]
                        )
                        out.append(nop)
                    inst.sync_info = mybir.SyncInfo(
                        on_wait=keep,
                        on_update=list(si.on_update) if si.on_update else [],
                    )
                out.append(inst)
            if changed:
                b.instructions = out


def _build():
    import concourse.bass as bass
    import concourse.mybir as mybir
    import concourse.tile as tile
    from concourse.masks import make_identity
    from collections import deque

    f32 = mybir.dt.float32
    f32r = mybir.dt.float32r
    bf16 = mybir.dt.bfloat16
    fp8 = mybir.dt.float8e4
    i32 = mybir.dt.int32
    DR = mybir.MatmulPerfMode.DoubleRow
    SCHG = int(os.environ.get("SCHG", "0"))

    nc = bass.Bass(trn_type="TRN2")

    xb_d = nc.dram_tensor("xb", [DIM, N], bf16, kind="ExternalInput")
    x8_d = nc.dram_tensor("x8", [DIM, N], fp8, kind="ExternalInput")
    w1_d = nc.dram_tensor("w1", [DIM, 2 * DIM], bf16, kind="ExternalInput")
    w28_d = nc.dram_tensor("w28", [DIM, 2 * DIM], fp8, kind="ExternalInput")
    wv1_d = nc.dram_tensor("wv1b", [DIM, DIM], bf16, kind="ExternalInput")
    wp_d = nc.dram_tensor("wp", [DIM, DIM], bf16, kind="ExternalInput")
    bp_d = nc.dram_tensor("bp", [DIM], f32, kind="ExternalInput")
    lam_d = nc.dram_tensor("lam", [H], f32, kind="ExternalInput")
    out_d = nc.dram_tensor("out", [NQ, DIM], f32, kind="ExternalOutput")
    # DRAM bounce scratch for the q2/k2 partition-fold relayout
    # (alternating by pair parity so same-queue FIFO orders reuse)
    k2b_d = [
        nc.dram_tensor(f"k2b{i}", [128, N], fp8, kind="Internal")
        for i in range(2)
    ]
    q2b_d = [
        nc.dram_tensor(f"q2b{i}", [128, NQ], fp8, kind="Internal")
        for i in range(2)
    ]

    C = 6          # 768 / 128 feature chunks
    NPAIR = 6      # head pairs
    TT = 16        # token tiles of 128 in N
    QT = 4         # query sub-tiles of 128 in NQ

    with tile.TileContext(nc) as tc:
        with (
            tc.tile_pool(name="persist", bufs=1) as pp,
            tc.tile_pool(name="psum", bufs=1, space="PSUM") as psp,
        ):
            # ---- constants / small tiles ----
            ident = pp.tile([128, 128], f32, tag="ident")
            make_identity(nc, ident[:])
            lam_b = pp.tile([128, H], f32, tag="lam_b")
            nc.gpsimd.dma_start(
                out=lam_b[:],
                in_=bass.AP(tensor=lam_d, offset=0, ap=[[0, 128], [1, H]]),
            )
            bp_row = pp.tile([1, DIM], f32, tag="bp_row")
            nc.gpsimd.dma_start(
                out=bp_row[:],
                in_=bass.AP(tensor=bp_d, offset=0, ap=[[0, 1], [1, DIM]]),
            )
            ones_col = pp.tile([1, 128], f32, tag="ones_col")
            nc.vector.memset(ones_col[:], 1.0)
            esh_t = pp.tile([128, 1], f32, tag="esh_t")
            nc.vector.memset(esh_t[:], -ESH)

            # ---- persistent big tiles ----
            xT8 = pp.tile([128, C, N], fp8, tag="xT8")
            xTb = pp.tile([128, C, N], bf16, tag="xTb")
            Y = pp.tile([128, QT, DIM], f32, tag="Y")
            yT = pp.tile([128, C, NQ], bf16, tag="yT")
            stats = pp.tile([128, QT, C + 1, 6], f32, tag="stats")
            wpj = pp.tile([128, C, DIM], bf16, tag="wpj")

            def dma_xTb(lo, hi):
                nc.sync.dma_start(
                    xTb[:, :, lo:hi],
                    xb_d[:, lo:hi].rearrange("(c p) m -> p c m", p=128),
                )

            def dma_xT8(lo, hi):
                nc.sync.dma_start(
                    xT8[:, :, lo:hi],
                    x8_d[:, lo:hi].rearrange("(c p) m -> p c m", p=128),
                )

            with (
                tc.tile_pool(name="pairs", bufs=2) as wpool,
                tc.tile_pool(name="epool", bufs=int(os.environ.get("EPBUFS", "17"))) as ep,
                tc.tile_pool(name="schpool", bufs=2) as schp,
            ):
                # ---- weight slice DMA + GEMM emit helpers ----
                def dma_wslice(tag, src_w, col0, dt=bf16):
                    t = wpool.tile([128, C, 128], dt, tag=tag, name=tag)
                    nc.sync.dma_start(
                        t[:],
                        src_w[:, col0 : col0 + 128].rearrange(
                            "(c p2) n -> p2 c n", p2=128
                        ),
                    )
                    return t

                def dma_w8slice(tag, col0):
                    # fp8 DoubleRow layout: [part, chunk-pair, row-half, col]
                    t = wpool.tile([128, 3, 2, 128], fp8, tag=tag, name=tag)
                    nc.sync.dma_start(
                        t[:],
                        w28_d[:, col0 : col0 + 128].rearrange(
                            "(cc two p2) n -> p2 cc two n", p2=128, two=2
                        ),
                    )
                    return t

                def alloc_q(tag):
                    return wpool.tile([128, NQ], bf16, tag=tag, name=tag)

                def emit_q_fill(qp, wq, lo=0, hi=NQ):
                    ps = psp.tile([128, 512], f32, tag="mm", bufs=2, name="psq")
                    w = hi - lo
                    for c in range(C):
                        nc.tensor.matmul(
                            ps[:, 0:w],
                            wq[:, c, :],
                            xTb[:, c, lo:hi],
                            start=(c == 0),
                            stop=(c == C - 1),
                        )
                    nc.vector.tensor_copy(qp[:, lo:hi], ps[:, 0:w])

                def alloc_k(tag):
                    return wpool.tile([128, N], bf16, tag=tag, name=tag)

                def emit_k_gemm(kt, wk, mt):
                    ps = psp.tile([128, 512], f32, tag="mm", bufs=2, name="psk")
                    for c in range(C):
                        nc.tensor.matmul(
                            ps[:],
                            wk[:, c, :],
                            xTb[:, c, mt * 512 : (mt + 1) * 512],
                            start=(c == 0),
                            stop=(c == C - 1),
                        )
                    nc.vector.tensor_copy(kt[:, mt * 512 : (mt + 1) * 512], ps[:])

                # fp8 q2/k2: GEMM -> fp8 staging tile -> DRAM bounce ->
                # [32, head, d-half, tok] fold for DoubleRow QK2
                def emit_q_fill8(q2f8, wq8, par):
                    ps = psp.tile([128, 512], f32, tag="mm", bufs=2, name="psq8")
                    for cc in range(3):
                        nc.tensor.matmul(
                            ps[:],
                            wq8[:, cc],
                            xT8[:, 2 * cc : 2 * cc + 2, 0:NQ],
                            start=(cc == 0),
                            stop=(cc == 2),
                            perf_mode=DR,
                        )
                    nc.vector.tensor_copy(q2f8[:], ps[:])
                    nc.sync.dma_start(q2b_d[par][:], q2f8[:])

                def emit_k_gemm8(k2f8, wk8, mt, par):
                    ps = psp.tile([128, 512], f32, tag="mm", bufs=2, name="psk8")
                    for cc in range(3):
                        nc.tensor.matmul(
                            ps[:],
                            wk8[:, cc],
                            xT8[:, 2 * cc : 2 * cc + 2, mt * 512 : (mt + 1) * 512],
                            start=(cc == 0),
                            stop=(cc == 2),
                            perf_mode=DR,
                        )
                    sl = slice(mt * 512, (mt + 1) * 512)
                    nc.vector.tensor_copy(k2f8[:, sl], ps[:])
                    nc.sync.dma_start(k2b_d[par][:, sl], k2f8[:, sl])

                def emit_k2_fold(k28t, par, lo=0, hi=N):
                    for h in range(2):
                        nc.sync.dma_start(
                            k28t[:, h, :, lo:hi],
                            k2b_d[par][64 * h : 64 * h + 64, lo:hi].rearrange(
                                "(two p) n -> p two n", p=32
                            ),
                        )

                def emit_q2_fold(q28t, par):
                    for h in range(2):
                        nc.sync.dma_start(
                            q28t[:, h],
                            q2b_d[par][64 * h : 64 * h + 64, :].rearrange(
                                "(two p) n -> p two n", p=32
                            ),
                        )

                def alloc_v1():
                    v = wpool.tile(
                        [128, TT, 2, HD + 1], bf16, tag="v1p", name="v1p"
                    )
                    nc.vector.memset(v[:, :, :, HD : HD + 1], 1.0)
                    return v

                def alloc_v18():
                    v = wpool.tile(
                        [128, TT, 2, HD + 1], fp8, tag="v18", name="v18"
                    )
                    nc.vector.memset(v[:, :, :, HD : HD + 1], 1.0)
                    return v

                def emit_v1_unit(vdst, v8dst, wv, u):
                    # token tiles 2u, 2u+1; out token-major [tok, 2*HD]
                    ps = psp.tile([128, 512], f32, tag="mm", bufs=2, name="psv")
                    for half in range(2):
                        t = 2 * u + half
                        for c in range(C):
                            nc.tensor.matmul(
                                ps[:, half * 128 : (half + 1) * 128],
                                xTb[:, c, t * 128 : (t + 1) * 128],
                                wv[:, c, :],
                                start=(c == 0),
                                stop=(c == C - 1),
                            )
                    nc.vector.tensor_copy(
                        vdst[:, 2 * u : 2 * u + 2, :, 0:HD],
                        ps[:, 0:256].rearrange("p (t h d) -> p t h d", t=2, h=2),
                    )
                    nc.vector.tensor_copy(
                        v8dst[:, 2 * u : 2 * u + 2, :, 0:HD],
                        ps[:, 0:256].rearrange("p (t h d) -> p t h d", t=2, h=2),
                    )

                # ---- proj stage 1: accumulate chunks 0-4 of the proj
                # GEMM into an SBUF accumulator (runs as late deferred
                # units inside pair 5); chunk 5 joins at the tail ----
                pacc = pp.tile([128, QT, DIM], f32, tag="pacc")

                def emit_proj_stage1(j, half):
                    # chunks 0-4 of y@Wp, plus bias via rank-1 ones x bp
                    jr = slice(j * 128, (j + 1) * 128)
                    hs = slice(half * 384, (half + 1) * 384)
                    ps = psp.tile([128, 512], f32, tag="mm", bufs=2, name="ps1")
                    nc.tensor.matmul(
                        ps[:, 0:384],
                        ones_col[:, 0:128],
                        bp_row[:, hs],
                        start=True,
                        stop=False,
                    )
                    for c in range(5):
                        nc.tensor.matmul(
                            ps[:, 0:384],
                            yT[:, c, jr],
                            wpj[:, c, hs],
                            start=False,
                            stop=(c == 4),
                        )
                    nc.vector.tensor_copy(pacc[:, j, hs], ps[:, 0:384])

                # ---- per-head finish for chunk 5 (heads 10, 11) ----
                def emit_head_finish_j(h, j, tail=False):
                    hh = h % 2
                    tp = psp.tile([128, 512], f32, tag="mm", bufs=2, name="tp")
                    nc.tensor.transpose(
                        tp[0:64, 0:128],
                        Y[:, j, h * 64 : (h + 1) * 64],
                        ident[:],
                    )
                    # in the tail ACT is idle (exp stream done) while DVE
                    # carries the serial rms chain — route the copy there
                    cp = nc.scalar.copy if tail else nc.vector.tensor_copy
                    cp(
                        yT[hh * 64 : (hh + 1) * 64, 5, j * 128 : (j + 1) * 128],
                        tp[0:64, 0:128],
                    )
                    nc.vector.bn_stats(
                        out=stats[:, j, 5 + hh, :],
                        in_=Y[:, j, h * 64 : (h + 1) * 64],
                    )

                def emit_head_finish(h):
                    for j in range(QT):
                        emit_head_finish_j(h, j)

                # ---- chunk finish: transpose Y chunk + bn stats ----
                def emit_chunk_finish_j(cc, j):
                    tp = psp.tile([128, 512], f32, tag="mm", bufs=2, name="tp")
                    nc.tensor.transpose(
                        tp[:, 0:128],
                        Y[:, j, cc * 128 : (cc + 1) * 128],
                        ident[:],
                    )
                    nc.vector.tensor_copy(
                        yT[:, cc, j * 128 : (j + 1) * 128], tp[:, 0:128]
                    )
                    nc.vector.bn_stats(
                        out=stats[:, j, cc, :],
                        in_=Y[:, j, cc * 128 : (cc + 1) * 128],
                    )

                def emit_chunk_finish(cc):
                    for j in range(QT):
                        emit_chunk_finish_j(cc, j)

                # ---- one-shot work units + paced global queue ----
                # Every prefetched GEMM is a one-shot closure. The pair
                # that OWNS a unit force-runs it at the latest safe point
                # (deadline hooks inside its own chains); paced pops from
                # the global queue run units early when PE has slack, so
                # leftover work naturally flows into the last pairs where
                # the exp stream would otherwise throttle PE.
                def unit(fn):
                    st = [False]

                    def run():
                        if st[0]:
                            return False
                        st[0] = True
                        fn()
                        return True

                    run.consume = lambda: st.__setitem__(0, True)
                    return run

                global_q = deque()
                defer_q = deque()

                def pop1(_=None):
                    while global_q:
                        if global_q.popleft()():
                            return
                    while defer_q:
                        if defer_q.popleft()():
                            return

                def drain_all():
                    while global_q:
                        global_q.popleft()()
                    while defer_q:
                        defer_q.popleft()()

                PACE = int(os.environ.get("PACE", "1"))

                class PU:
                    pass

                def make_pair_units(wq1s, wk1s, wq2s8, wk2s8, wv1s,
                                    q1t, k1t, v1t, v8t,
                                    q2f8t, k2f8t, q28t, k28t, par,
                                    piecewise=False):
                    pu = PU()
                    pu.q1 = unit(lambda: emit_q_fill(q1t, wq1s))
                    pu.k1 = [
                        unit(lambda m=m: emit_k_gemm(k1t, wk1s, m))
                        for m in range(4)
                    ]
                    pu.v1 = [
                        unit(lambda u=u: emit_v1_unit(v1t, v8t, wv1s, u))
                        for u in range(8)
                    ]

                    def q2fn():
                        emit_q_fill8(q2f8t, wq2s8, par)
                        if piecewise:
                            emit_q2_fold(q28t, par)

                    def k2fn(m):
                        def f():
                            emit_k_gemm8(k2f8t, wk2s8, m, par)
                            if piecewise:
                                # same-queue FIFO: out(m) precedes in(m)
                                emit_k2_fold(k28t, par,
                                             m * 512, (m + 1) * 512)

                        return f

                    pu.q2 = unit(q2fn)
                    pu.k2 = [unit(k2fn(m)) for m in range(4)]
                    if piecewise:
                        pu.k2f = unit(lambda: None)
                        pu.q2f = unit(lambda: None)
                    else:
                        pu.k2f = unit(lambda: emit_k2_fold(k28t, par))
                        pu.q2f = unit(lambda: emit_q2_fold(q28t, par))
                    return pu

                def c1_qk_hook(pu, jit=False):
                    def h(g):
                        pu.q1()
                        for m in range(g // 2 + 1):
                            pu.k1[m]()
                        if jit:
                            # pair 0: wv1/x8/w28 DMAs land after xTb; forcing
                            # v1/k2 here would stall the in-order PE on DMA
                            # waits while QK work is ready. v1 is forced
                            # across the 4th QK phase; k2/q2/folds at c2 g0.
                            return
                        if g >= 2:
                            pu.v1[g - 2]()
                        # fp8 k2/q2 pipeline: GEMMs + bounce-out by g==7,
                        # fold-in DMAs issued right after (chain2 needs the
                        # folded tiles before its first QK matmul)
                        if g == 1:
                            pu.q2()
                        if g % 2 == 1:
                            pu.k2[(g - 1) // 2]()
                        if g == 7:
                            pu.k2f()
                            pu.q2f()
                        if qk_pop_ok(g):
                            pop1()

                    return h

                def c1_av_hook(pu):
                    def h(j):
                        if j == 0:
                            for u in pu.v1:
                                u()
                        else:
                            pop1()

                    return h

                def c2_qk_hook(pu):
                    def h(g):
                        if g == 0:
                            # safety: idempotent if already forced in chain1
                            pu.q2()
                            for u in pu.k2:
                                u()
                            pu.k2f()
                            pu.q2f()
                        if g % PACE == 1 and not (
                            qk_nodefer[0] and not global_q
                        ):
                            pop1()

                    return h

                QKPOP = os.environ.get("QKPOP", "none")

                def qk_pop_ok(g):
                    if QKPOP == "none":
                        return False
                    if QKPOP == "g7":
                        return g == 7
                    if QKPOP == "odd":
                        return g % 2 == 1
                    return True

                DEFPOP = int(os.environ.get("DEFPOP", "1"))
                # guard: during the LAST pair's QK phases the global queue
                # is empty and every pop would stuff a ~1us deferred unit
                # (proj stage1 etc.) between QK groups, stalling the
                # ACT-critical exp stream; hold them for the AV phases
                qk_nodefer = [False]

                def paced_qk(g):
                    if qk_pop_ok(g):
                        pop1()
                    elif DEFPOP and defer_q and not global_q \
                            and not qk_nodefer[0]:
                        pop1()

                AVJ = int(os.environ.get("AVJ", "1"))

                def paced_av(j):
                    if j >= AVJ:
                        pop1()

                # ---- attention chain, split into QK and AV phases so
                # chain2's QK (and its exp stream) overlaps chain1's AV ----
                def qk_phase(hh, kt, qp, qk_hook=None,
                             fp8av=False, k8q8=None):
                    po = hh * HD
                    ets = []
                    for g in range(8):
                        if qk_hook is not None:
                            qk_hook(g)
                        qk = psp.tile(
                            [128, 2, 512], f32, tag="qk", bufs=2, name="qk"
                        )
                        for g2 in range(2):
                            mc = g * 2 + g2
                            if k8q8 is not None:
                                k28t, q28t = k8q8
                                nc.tensor.matmul(
                                    qk[:, g2, :],
                                    k28t[:, hh, :, mc * 128 : (mc + 1) * 128],
                                    q28t[:, hh],
                                    start=True,
                                    stop=True,
                                    perf_mode=DR,
                                )
                            else:
                                nc.tensor.matmul(
                                    qk[:, g2, :],
                                    kt[po : po + 64, mc * 128 : (mc + 1) * 128],
                                    qp[po : po + 64, :],
                                    start=True,
                                    stop=True,
                                )
                        if fp8av:
                            e_t = ep.tile(
                                [128, 2, 512], fp8, tag="E8", name="e8_t"
                            )
                            nc.scalar.activation(
                                e_t[:],
                                qk[:],
                                mybir.ActivationFunctionType.Exp,
                                bias=esh_t[:],
                                scale=0.125,
                            )
                        else:
                            e_t = ep.tile(
                                [128, 2, 512], bf16, tag="E", name="e_t"
                            )
                            nc.scalar.activation(
                                e_t[:],
                                qk[:],
                                mybir.ActivationFunctionType.Exp,
                                scale=0.125,
                            )
                        ets.append(e_t)
                    return ets

                def av_phase(hh, v1p, ets, av_hook=None, fp8av=False):
                    av = psp.tile(
                        [128, QT, HD + 1], f32, tag="av", bufs=2, name="av"
                    )
                    for j in range(QT):
                        if av_hook is not None:
                            av_hook(j)
                        if fp8av:
                            for g in range(8):
                                nc.tensor.matmul(
                                    av[:, j, :],
                                    ets[g][:, :, j * 128 : (j + 1) * 128],
                                    v1p[:, 2 * g : 2 * g + 2, hh, :],
                                    start=(g == 0),
                                    stop=(g == 7),
                                    perf_mode=DR,
                                )
                        else:
                            for mc in range(16):
                                nc.tensor.matmul(
                                    av[:, j, :],
                                    ets[mc // 2][:, mc % 2, j * 128 : (j + 1) * 128],
                                    v1p[:, mc, hh, :],
                                    start=(mc == 0),
                                    stop=(mc == 15),
                                )
                    return av

                def combine(h, av1, av2):
                    # reads the AV psums directly; av1 released first so
                    # the next chain can reuse the psum buffer sooner
                    r1 = wpool.tile([128, QT, 1], f32, tag="r1", bufs=2, name="r1")
                    nc.vector.reciprocal(r1[:], av1[:, :, HD : HD + 1])
                    t1 = wpool.tile([128, QT, HD], f32, tag="t1", bufs=1, name="t1")
                    r1b = bass.AP(
                        tensor=r1.tensor,
                        offset=r1.offset,
                        ap=[r1.ap[0], r1.ap[1], [0, HD]],
                    )
                    nc.vector.tensor_tensor(
                        out=t1[:],
                        in0=av1[:, :, 0:HD],
                        in1=r1b,
                        op=mybir.AluOpType.mult,
                    )
                    r2 = wpool.tile([128, QT, 1], f32, tag="r2", bufs=2, name="r2")
                    nc.vector.reciprocal(r2[:], av2[:, :, HD : HD + 1])
                    lam_h = lam_b[:, h : h + 1]
                    lam_bc = bass.AP(
                        tensor=lam_h.tensor,
                        offset=lam_h.offset,
                        ap=[lam_h.ap[0], [0, QT], [0, 1]],
                    )
                    nc.vector.tensor_tensor(
                        out=r2[:], in0=r2[:], in1=lam_bc, op=mybir.AluOpType.mult
                    )
                    t2 = wpool.tile([128, QT, HD], f32, tag="t2", bufs=1, name="t2")
                    r2b = bass.AP(
                        tensor=r2.tensor,
                        offset=r2.offset,
                        ap=[r2.ap[0], r2.ap[1], [0, HD]],
                    )
                    nc.vector.tensor_tensor(
                        out=t2[:],
                        in0=av2[:, :, 0:HD],
                        in1=r2b,
                        op=mybir.AluOpType.mult,
                    )
                    nc.vector.tensor_tensor(
                        out=Y[:, :, h * 64 : (h + 1) * 64],
                        in0=t1[:],
                        in1=t2[:],
                        op=mybir.AluOpType.subtract,
                    )

                # ---- startup DMAs (issue order = DMA device order) ----
                # chain1 QK of pair 0 paces the whole ramp: its xTb/w1
                # stream goes first; wv1 (needed at c1AV) and the fp8 side
                # (needed at c2QK, ~8 exp-groups later) follow
                dma_xTb(0, 256)
                wq1 = dma_wslice("wq1", w1_d, 0)
                dma_xTb(256, 512)
                wk1 = dma_wslice("wk1", w1_d, DIM)
                dma_xTb(512, 1024)
                dma_xTb(1024, 1536)
                dma_xTb(1536, 2048)
                dma_xT8(0, 1024)
                wq2 = dma_w8slice("wq2", 0)
                wk2 = dma_w8slice("wk2", DIM)
                dma_xT8(1024, 2048)
                wv1 = dma_wslice("wv1", wv1_d, 0, bf16)

                def alloc_pair_fp8():
                    q2f8t = wpool.tile([128, NQ], fp8, tag="q2f8", name="q2f8")
                    k2f8t = wpool.tile([128, N], fp8, tag="k2f8", name="k2f8")
                    q28t = wpool.tile([32, 2, 2, NQ], fp8, tag="q28", name="q28")
                    k28t = wpool.tile([32, 2, 2, N], fp8, tag="k28", name="k28")
                    return q2f8t, k2f8t, q28t, k28t

                # ---- pair-0 prologue ----
                q1p = alloc_q("q1p")
                emit_q_fill(q1p, wq1, 0, 256)
                emit_q_fill(q1p, wq1, 256, 512)
                k1T = alloc_k("k1T")
                v1p = alloc_v1()
                v18p = alloc_v18()
                q2f8, k2f8, q28, k28 = alloc_pair_fp8()
                pu = make_pair_units(wq1, wk1, wq2, wk2, wv1,
                                     q1p, k1T, v1p, v18p,
                                     q2f8, k2f8, q28, k28, par=0,
                                     piecewise=True)
                pu.q1.consume()  # filled above in halves
                pu.k1[0]()

                # ---- pair loop ----
                for p in range(NPAIR):
                    nx = p + 1
                    if nx < NPAIR:
                        wq1n = dma_wslice("wq1", w1_d, nx * 128)
                        wk1n = dma_wslice("wk1", w1_d, DIM + nx * 128)
                        wq2n = dma_w8slice("wq2", nx * 128)
                        wk2n = dma_w8slice("wk2", DIM + nx * 128)
                        wv1n = dma_wslice("wv1", wv1_d, nx * 128, bf16)
                        if nx == 4:
                            nc.sync.dma_start(
                                wpj[:],
                                wp_d[:, :].rearrange("(c p) n -> p c n", p=128),
                            )
                        q1pn = alloc_q("q1p")
                        k1Tn = alloc_k("k1T")
                        v1pn = alloc_v1()
                        v18pn = alloc_v18()
                        q2f8n, k2f8n, q28n, k28n = alloc_pair_fp8()
                        pun = make_pair_units(wq1n, wk1n, wq2n, wk2n, wv1n,
                                              q1pn, k1Tn, v1pn, v18pn,
                                              q2f8n, k2f8n, q28n, k28n,
                                              par=nx % 2)
                        global_q.append(pun.q1)
                        for u in pun.k1:
                            global_q.append(u)
                        for u in pun.v1:
                            global_q.append(u)
                        global_q.append(pun.q2)
                        for u in pun.k2:
                            global_q.append(u)
                        global_q.append(pun.k2f)
                        global_q.append(pun.q2f)
                        if p >= 1:
                            defer_q.append(
                                unit(lambda cc=p - 1: emit_chunk_finish(cc))
                            )
                    elif p == NPAIR - 1:
                        defer_q.append(unit(lambda: emit_chunk_finish(4)))
                        for j in range(QT):
                            for half in range(2):
                                defer_q.append(
                                    unit(
                                        lambda jj=j, hh=half: emit_proj_stage1(
                                            jj, hh
                                        )
                                    )
                                )

                    # all four QK phases first: 16 bf16 exps of run-ahead
                    # cover the fp8 side's GEMM+bounce+fold latency, and
                    # the exp stream never starves at phase boundaries
                    qk_nodefer[0] = p == NPAIR - 1

                    def p0_c2b_hook(g, pu=pu):
                        pu.v1[g]()  # wv1 has landed by the 4th QK phase
                        paced_qk(g)

                    ets1a = qk_phase(0, k1T, q1p,
                                     qk_hook=c1_qk_hook(pu, jit=(p == 0)))
                    ets1b = qk_phase(1, k1T, q1p, qk_hook=paced_qk)
                    ets2a = qk_phase(0, None, None, qk_hook=c2_qk_hook(pu),
                                     fp8av=True, k8q8=(k28, q28))
                    ets2b = qk_phase(1, None, None,
                                     qk_hook=(p0_c2b_hook if p == 0
                                              else paced_qk),
                                     fp8av=True, k8q8=(k28, q28))
                    qk_nodefer[0] = False
                    av1 = av_phase(0, v1p, ets1a, av_hook=c1_av_hook(pu))
                    av2 = av_phase(0, v18p, ets2a, av_hook=paced_av,
                                   fp8av=True)
                    combine(2 * p, av1, av2)
                    if p == NPAIR - 1:
                        defer_q.append(unit(lambda: emit_head_finish(10)))
                    av1 = av_phase(1, v1p, ets1b, av_hook=paced_av)
                    av2 = av_phase(1, v18p, ets2b, av_hook=paced_av,
                                   fp8av=True)
                    combine(2 * p + 1, av1, av2)

                    if nx < NPAIR:
                        k1T, v1p, v18p = k1Tn, v1pn, v18pn
                        q1p = q1pn
                        q28, k28 = q28n, k28n
                        pu = pun

                drain_all()

            # ---- tail: per query tile: chunk-5 finish -> rms -> proj ----
            with tc.tile_pool(name="proj", bufs=1) as prj:
                mv = prj.tile([128, QT, 2], f32, tag="mv")
                rms = prj.tile([128, QT], f32, tag="rms")
                eps_t = prj.tile([128, 1], f32, tag="eps_t")
                nc.vector.memset(eps_t[:], EPS)

                def emit_rms_j(j):
                    nc.vector.bn_aggr(out=mv[:, j, :], in_=stats[:, j])

                    nc.vector.tensor_tensor(
                        out=mv[:, j, 0:1],
                        in0=mv[:, j, 0:1],
                        in1=mv[:, j, 0:1],
                        op=mybir.AluOpType.mult,
                    )
                    nc.vector.tensor_tensor(
                        out=mv[:, j, 1:2],
                        in0=mv[:, j, 1:2],
                        in1=mv[:, j, 0:1],
                        op=mybir.AluOpType.add,
                    )
                    nc.scalar.activation(
                        rms[:, j : j + 1],
                        mv[:, j, 1:2],
                        mybir.ActivationFunctionType.Sqrt,
                        bias=eps_t[:],
                        scale=1.0,
                    )
                    nc.vector.reciprocal(rms[:, j : j + 1], rms[:, j : j + 1])

                def emit_proj_j(j):
                    # stage 2: chunk-5 contribution joins the stage-1
                    # accumulator; rms folds into the epilogue:
                    # (y*rms) @ Wp = (y @ Wp) * rms  (rms is per-token)
                    jr = slice(j * 128, (j + 1) * 128)
                    osb2 = prj.tile(
                        [128, DIM], f32, tag="out_sb", bufs=2, name="osb2"
                    )
                    osb3 = prj.tile(
                        [128, DIM], f32, tag="out_sb3", bufs=2, name="osb3"
                    )
                    for half in range(2):
                        ps = psp.tile([128, 512], f32, tag="mm", bufs=2, name="psp2")
                        nc.tensor.matmul(
                            ps[:, 0:384],
                            yT[:, 5, jr],
                            wpj[:, 5, half * 384 : (half + 1) * 384],
                            start=True,
                            stop=True,
                        )
                        hs = slice(half * 384, (half + 1) * 384)
                        nc.vector.tensor_tensor(
                            out=osb2[:, hs],
                            in0=pacc[:, j, hs],
                            in1=ps[:, 0:384],
                            op=mybir.AluOpType.add,
                        )
                        # rms scale on ACT: the exp stream is finished by
                        # the tail, so ACT is free while DVE is the serial
                        # bottleneck of the rms/epilogue chain
                        nc.scalar.activation(
                            osb3[:, hs],
                            osb2[:, hs],
                            mybir.ActivationFunctionType.Copy,
                            scale=rms[:, j : j + 1],
                        )
                        nc.sync.dma_start(
                            out_d[j * 128 : (j + 1) * 128, hs], osb3[:, hs]
                        )

                for j in range(QT):
                    emit_head_finish_j(11, j, tail=True)
                    emit_rms_j(j)
                    emit_proj_j(j)

    _split_waits(nc)
    return nc


def kernel(x, W_qkv1, W_qkv2, W_proj, b_proj, norm_w, lambda_1, lambda_2, xpos):
    import ml_dtypes
    from concourse.bass_utils import run_bass_kernel_spmd

    if "nc" not in _cache:
        _cache["nc"] = _build()
    nc = _cache["nc"]

    bf16 = ml_dtypes.bfloat16
    e4m3 = ml_dtypes.float8_e4m3
    x = np.asarray(x, dtype=np.float32)
    w1 = np.asarray(W_qkv1, dtype=np.float32)
    w2 = np.asarray(W_qkv2, dtype=np.float32)
    w1qk = np.ascontiguousarray(w1[:, : 2 * DIM]).astype(bf16)
    w28 = np.clip(w2[:, : 2 * DIM], -240, 240).astype(e4m3)
    wv1b = np.ascontiguousarray(w1[:, 2 * DIM :]).astype(bf16)
    wpb = np.ascontiguousarray(
        np.asarray(norm_w, dtype=np.float32)[:, None]
        * np.asarray(W_proj, dtype=np.float32)
    ).astype(bf16)
    bp = np.ascontiguousarray(np.asarray(b_proj, dtype=np.float32))
    lam = np.ascontiguousarray(
        (
            np.asarray(lambda_1, dtype=np.float32)
            - np.asarray(lambda_2, dtype=np.float32)
            + LAMBDA_INIT
        ).astype(np.float32)
    )

    in_maps = []
    for c in range(NCORES):
        b, qi = c // 4, c % 4
        # rotate tokens so this core's query block comes first; attention
        # is permutation-invariant over keys so only q/out order matters
        xr = np.roll(x[b], -qi * NQ, axis=0)
        xrT = np.ascontiguousarray(xr.T)
        in_maps.append(
            {
                "xb": xrT.astype(bf16),
                "x8": np.clip(xrT, -240, 240).astype(e4m3),
                "w1": w1qk,
                "w28": w28,
                "wv1b": wv1b,
                "wp": wpb,
                "bp": bp,
                "lam": lam,
            }
        )

    res = run_bass_kernel_spmd(nc, in_maps, core_ids=list(range(NCORES)))
    out = np.empty((B, N, DIM), dtype=np.float32)
    for c in range(NCORES):
        b, qi = c // 4, c % 4
        out[b, qi * NQ : (qi + 1) * NQ, :] = res.results[c]["out"]
    return out



# revision 73
# speedup vs baseline: 1.2038x; 1.0033x over previous
"""DIFF-Attention Trainium2 kernel (v4: fp8 DoubleRow attn2 path).

Problem: B=2, N=2048, DIM=768, H=12, HD=64, two qkv projections, two
softmax attention maps, diff = attn1 - lam*attn2, out = diff @ v1,
RMSNorm, proj.

v4: the attn2 branch enters the output scaled by lam = l1-l2+0.1 ~=
0.108, so its quantization noise is suppressed ~9x. Exploit with
fp8e4m3 + MatmulPerfMode.DoubleRow (2 contraction rows packed per
matmul at 0.5 cycles/row) on: the k2/q2 qkv GEMMs (host-cast x8/w28
inputs), and AV2 (E2 = exp(S2*0.125 - ESH) written fp8 by the
activation, v18 = fp8 copy of v1). ESH shifts exp output under
e4m3's +-240 range (softmax is shift-invariant). QK2 stays bf16:
feeding it fp8 would need a [32,2,free] relayout that costs more
DVE than the PE it saves.

Sharding: 8 cores; core c handles batch b = c//4 and query tokens
[512*(c%4), 512*(c%4)+512). Attention is permutation-invariant over
keys, so the host hands each core x^T with the token axis ROTATED so
that the core's 512 query tokens come first: q GEMMs read the first
512 columns of the same xT tile the k/v GEMMs read (no separate xq
DMA). Each core computes k1/k2/v1 for the whole (permuted) batch and
q/attention/norm/proj only for its 512 query tokens. No collectives.

Structure:
  - Each attention chain runs its 8 QK+exp groups first (8 E tiles
    buffered), then 4 query-tile AV passes with out[query, hd]
    (queries on psum partitions, 65-wide moving dim). Sequential
    per-region psum accumulation — interleaved start/stop groups
    within one psum bank silently corrupt results.
  - combine() reads the AV psums directly (no PE transpose of O).
  - v1 is computed per head-pair (prefetched like k/q) as a bf16
    token-major GEMM from an on-chip bf16 copy of x^T.
  - Work units (next-pair k/q/v GEMMs, prev-chunk Y transpose+stats)
    are popped one per attention group. Units only needed by LATER
    chains (q2/k2 fills, chunk finish) are carryable across the pair
    boundary so the last pairs keep PE fed.
  - Tail pipelines per query tile: Y-chunk-5 transpose -> rms -> proj
    -> epilogue -> half-row output DMAs.

Numerics: f32r (tf32-like) qkv/QK GEMMs, bf16 exp(S) / AV / v1 / proj,
fp32 PSUM accumulation, RMSNorm stats in fp32.
"""

import os
import numpy as np

B, N, DIM, H, HD = 2, 2048, 768, 12, 64
NQ = 512            # query tokens per core
LAMBDA_INIT = 0.1
EPS = 1e-6
NCORES = 8
ESH = 5.0           # exp shift for fp8 E2 (max observed logit ~8.7; e4m3 max 240)
_LOG2E = 1.4426950408889634
SCH_A = float(0.125 * _LOG2E * (1 << 23))
SCH_B = float((1 << 23) * (127.0 - ESH * _LOG2E) - 366393.0)

_cache = {}


def _split_waits(nc, max_waits=1):
    """The walrus build in this environment rejects instructions carrying
    more than one explicit sync wait. Hoist excess waits onto NoOps
    inserted just before, on the same engine (same-engine program order
    makes this semantically equivalent)."""
    import concourse.mybir as mybir

    ctr = 0
    for f in nc.m.functions:
        for b in f.blocks:
            out = []
            changed = False
            for inst in b.instructions:
                si = inst.sync_info
                waits = list(si.on_wait) if si is not None and si.on_wait else []
                if len(waits) > max_waits:
                    changed = True
                    keep = waits[-max_waits:]
                    excess = waits[:-max_waits]
                    for i in range(0, len(excess), max_waits):
                        ctr += 1
                        nop = mybir.InstNoOp(
                            name=f"I-waitsplit-{ctr}", ins=[], outs=[]
                        )
                        nop.engine = inst.engine
                        nop.sync_info = mybir.SyncInfo(
                            on_wait=excess[i : i + max_waits], on_update=[]
                        )
                        out.append(nop)
                    inst.sync_info = mybir.SyncInfo(
                        on_wait=keep,
                        on_update=list(si.on_update) if si.on_update else [],
                    )
                out.append(inst)
            if changed:
                b.instructions = out


def _build():
    import concourse.bass as bass
    import concourse.mybir as mybir
    import concourse.tile as tile
    from concourse.masks import make_identity
    from collections import deque

    f32 = mybir.dt.float32
    f32r = mybir.dt.float32r
    bf16 = mybir.dt.bfloat16
    fp8 = mybir.dt.float8e4
    i32 = mybir.dt.int32
    DR = mybir.MatmulPerfMode.DoubleRow
    SCHG = int(os.environ.get("SCHG", "0"))

    nc = bass.Bass(trn_type="TRN2")

    xb_d = nc.dram_tensor("xb", [DIM, N], bf16, kind="ExternalInput")
    x8_d = nc.dram_tensor("x8", [DIM, N], fp8, kind="ExternalInput")
    w1_d = nc.dram_tensor("w1", [DIM, 2 * DIM], bf16, kind="ExternalInput")
    w28_d = nc.dram_tensor("w28", [DIM, 2 * DIM], fp8, kind="ExternalInput")
    wv1_d = nc.dram_tensor("wv1b", [DIM, DIM], bf16, kind="ExternalInput")
    wp_d = nc.dram_tensor("wp", [DIM, DIM], bf16, kind="ExternalInput")
    bp_d = nc.dram_tensor("bp", [DIM], f32, kind="ExternalInput")
    lam_d = nc.dram_tensor("lam", [H], f32, kind="ExternalInput")
    out_d = nc.dram_tensor("out", [NQ, DIM], f32, kind="ExternalOutput")
    # DRAM bounce scratch for the q2/k2 partition-fold relayout
    # (alternating by pair parity so same-queue FIFO orders reuse)
    k2b_d = [
        nc.dram_tensor(f"k2b{i}", [128, N], fp8, kind="Internal")
        for i in range(2)
    ]
    q2b_d = [
        nc.dram_tensor(f"q2b{i}", [128, NQ], fp8, kind="Internal")
        for i in range(2)
    ]

    C = 6          # 768 / 128 feature chunks
    NPAIR = 6      # head pairs
    TT = 16        # token tiles of 128 in N
    QT = 4         # query sub-tiles of 128 in NQ

    with tile.TileContext(nc) as tc:
        with (
            tc.tile_pool(name="persist", bufs=1) as pp,
            tc.tile_pool(name="psum", bufs=1, space="PSUM") as psp,
        ):
            # ---- constants / small tiles ----
            ident = pp.tile([128, 128], f32, tag="ident")
            make_identity(nc, ident[:])
            lam_b = pp.tile([128, H], f32, tag="lam_b")
            nc.gpsimd.dma_start(
                out=lam_b[:],
                in_=bass.AP(tensor=lam_d, offset=0, ap=[[0, 128], [1, H]]),
            )
            bp_row = pp.tile([1, DIM], f32, tag="bp_row")
            nc.gpsimd.dma_start(
                out=bp_row[:],
                in_=bass.AP(tensor=bp_d, offset=0, ap=[[0, 1], [1, DIM]]),
            )
            ones_col = pp.tile([1, 128], f32, tag="ones_col")
            nc.vector.memset(ones_col[:], 1.0)
            esh_t = pp.tile([128, 1], f32, tag="esh_t")
            nc.vector.memset(esh_t[:], -ESH)

            # ---- persistent big tiles ----
            xT8 = pp.tile([128, C, N], fp8, tag="xT8")
            xTb = pp.tile([128, C, N], bf16, tag="xTb")
            Y = pp.tile([128, QT, DIM], f32, tag="Y")
            yT = pp.tile([128, C, NQ], bf16, tag="yT")
            stats = pp.tile([128, QT, C + 1, 6], f32, tag="stats")
            wpj = pp.tile([128, C, DIM], bf16, tag="wpj")

            def dma_xTb(lo, hi):
                nc.sync.dma_start(
                    xTb[:, :, lo:hi],
                    xb_d[:, lo:hi].rearrange("(c p) m -> p c m", p=128),
                )

            def dma_xT8(lo, hi):
                nc.sync.dma_start(
                    xT8[:, :, lo:hi],
                    x8_d[:, lo:hi].rearrange("(c p) m -> p c m", p=128),
                )

            with (
                tc.tile_pool(name="pairs", bufs=2) as wpool,
                tc.tile_pool(name="epool", bufs=int(os.environ.get("EPBUFS", "17"))) as ep,
                tc.tile_pool(name="schpool", bufs=2) as schp,
            ):
                # ---- weight slice DMA + GEMM emit helpers ----
                def dma_wslice(tag, src_w, col0, dt=bf16):
                    t = wpool.tile([128, C, 128], dt, tag=tag, name=tag)
                    nc.sync.dma_start(
                        t[:],
                        src_w[:, col0 : col0 + 128].rearrange(
                            "(c p2) n -> p2 c n", p2=128
                        ),
                    )
                    return t

                def dma_w8slice(tag, col0):
                    # fp8 DoubleRow layout: [part, chunk-pair, row-half, col]
                    t = wpool.tile([128, 3, 2, 128], fp8, tag=tag, name=tag)
                    nc.sync.dma_start(
                        t[:],
                        w28_d[:, col0 : col0 + 128].rearrange(
                            "(cc two p2) n -> p2 cc two n", p2=128, two=2
                        ),
                    )
                    return t

                def alloc_q(tag):
                    return wpool.tile([128, NQ], bf16, tag=tag, name=tag)

                def emit_q_fill(qp, wq, lo=0, hi=NQ):
                    ps = psp.tile([128, 512], f32, tag="mm", bufs=2, name="psq")
                    w = hi - lo
                    for c in range(C):
                        nc.tensor.matmul(
                            ps[:, 0:w],
                            wq[:, c, :],
                            xTb[:, c, lo:hi],
                            start=(c == 0),
                            stop=(c == C - 1),
                        )
                    nc.vector.tensor_copy(qp[:, lo:hi], ps[:, 0:w])

                def alloc_k(tag):
                    return wpool.tile([128, N], bf16, tag=tag, name=tag)

                def emit_k_gemm(kt, wk, mt):
                    ps = psp.tile([128, 512], f32, tag="mm", bufs=2, name="psk")
                    for c in range(C):
                        nc.tensor.matmul(
                            ps[:],
                            wk[:, c, :],
                            xTb[:, c, mt * 512 : (mt + 1) * 512],
                            start=(c == 0),
                            stop=(c == C - 1),
                        )
                    nc.vector.tensor_copy(kt[:, mt * 512 : (mt + 1) * 512], ps[:])

                # fp8 q2/k2: GEMM -> fp8 staging tile -> DRAM bounce ->
                # [32, head, d-half, tok] fold for DoubleRow QK2
                def emit_q_fill8(q2f8, wq8, par):
                    ps = psp.tile([128, 512], f32, tag="mm", bufs=2, name="psq8")
                    for cc in range(3):
                        nc.tensor.matmul(
                            ps[:],
                            wq8[:, cc],
                            xT8[:, 2 * cc : 2 * cc + 2, 0:NQ],
                            start=(cc == 0),
                            stop=(cc == 2),
                            perf_mode=DR,
                        )
                    nc.vector.tensor_copy(q2f8[:], ps[:])
                    nc.sync.dma_start(q2b_d[par][:], q2f8[:])

                def emit_k_gemm8(k2f8, wk8, mt, par):
                    ps = psp.tile([128, 512], f32, tag="mm", bufs=2, name="psk8")
                    for cc in range(3):
                        nc.tensor.matmul(
                            ps[:],
                            wk8[:, cc],
                            xT8[:, 2 * cc : 2 * cc + 2, mt * 512 : (mt + 1) * 512],
                            start=(cc == 0),
                            stop=(cc == 2),
                            perf_mode=DR,
                        )
                    sl = slice(mt * 512, (mt + 1) * 512)
                    nc.vector.tensor_copy(k2f8[:, sl], ps[:])
                    nc.sync.dma_start(k2b_d[par][:, sl], k2f8[:, sl])

                def emit_k2_fold(k28t, par, lo=0, hi=N):
                    for h in range(2):
                        nc.sync.dma_start(
                            k28t[:, h, :, lo:hi],
                            k2b_d[par][64 * h : 64 * h + 64, lo:hi].rearrange(
                                "(two p) n -> p two n", p=32
                            ),
                        )

                def emit_q2_fold(q28t, par):
                    for h in range(2):
                        nc.sync.dma_start(
                            q28t[:, h],
                            q2b_d[par][64 * h : 64 * h + 64, :].rearrange(
                                "(two p) n -> p two n", p=32
                            ),
                        )

                def alloc_v1():
                    v = wpool.tile(
                        [128, TT, 2, HD + 1], bf16, tag="v1p", name="v1p"
                    )
                    nc.vector.memset(v[:, :, :, HD : HD + 1], 1.0)
                    return v

                def alloc_v18():
                    v = wpool.tile(
                        [128, TT, 2, HD + 1], fp8, tag="v18", name="v18"
                    )
                    nc.vector.memset(v[:, :, :, HD : HD + 1], 1.0)
                    return v

                def emit_v1_unit(vdst, v8dst, wv, u):
                    # token tiles 2u, 2u+1; out token-major [tok, 2*HD]
                    ps = psp.tile([128, 512], f32, tag="mm", bufs=2, name="psv")
                    for half in range(2):
                        t = 2 * u + half
                        for c in range(C):
                            nc.tensor.matmul(
                                ps[:, half * 128 : (half + 1) * 128],
                                xTb[:, c, t * 128 : (t + 1) * 128],
                                wv[:, c, :],
                                start=(c == 0),
                                stop=(c == C - 1),
                            )
                    nc.vector.tensor_copy(
                        vdst[:, 2 * u : 2 * u + 2, :, 0:HD],
                        ps[:, 0:256].rearrange("p (t h d) -> p t h d", t=2, h=2),
                    )
                    nc.vector.tensor_copy(
                        v8dst[:, 2 * u : 2 * u + 2, :, 0:HD],
                        ps[:, 0:256].rearrange("p (t h d) -> p t h d", t=2, h=2),
                    )

                # ---- proj stage 1: accumulate chunks 0-4 of the proj
                # GEMM into an SBUF accumulator (runs as late deferred
                # units inside pair 5); chunk 5 joins at the tail ----
                pacc = pp.tile([128, QT, DIM], f32, tag="pacc")

                def emit_proj_stage1(j, half):
                    # chunks 0-4 of y@Wp, plus bias via rank-1 ones x bp
                    jr = slice(j * 128, (j + 1) * 128)
                    hs = slice(half * 384, (half + 1) * 384)
                    ps = psp.tile([128, 512], f32, tag="mm", bufs=2, name="ps1")
                    nc.tensor.matmul(
                        ps[:, 0:384],
                        ones_col[:, 0:128],
                        bp_row[:, hs],
                        start=True,
                        stop=False,
                    )
                    for c in range(5):
                        nc.tensor.matmul(
                            ps[:, 0:384],
                            yT[:, c, jr],
                            wpj[:, c, hs],
                            start=False,
                            stop=(c == 4),
                        )
                    nc.vector.tensor_copy(pacc[:, j, hs], ps[:, 0:384])

                # ---- per-head finish for chunk 5 (heads 10, 11) ----
                def emit_head_finish_j(h, j, tail=False):
                    hh = h % 2
                    tp = psp.tile([128, 512], f32, tag="mm", bufs=2, name="tp")
                    nc.tensor.transpose(
                        tp[0:64, 0:128],
                        Y[:, j, h * 64 : (h + 1) * 64],
                        ident[:],
                    )
                    # in the tail ACT is idle (exp stream done) while DVE
                    # carries the serial rms chain — route the copy there
                    cp = nc.scalar.copy if tail else nc.vector.tensor_copy
                    cp(
                        yT[hh * 64 : (hh + 1) * 64, 5, j * 128 : (j + 1) * 128],
                        tp[0:64, 0:128],
                    )
                    nc.vector.bn_stats(
                        out=stats[:, j, 5 + hh, :],
                        in_=Y[:, j, h * 64 : (h + 1) * 64],
                    )

                def emit_head_finish(h):
                    for j in range(QT):
                        emit_head_finish_j(h, j)

                # ---- chunk finish: transpose Y chunk + bn stats ----
                def emit_chunk_finish_j(cc, j):
                    tp = psp.tile([128, 512], f32, tag="mm", bufs=2, name="tp")
                    nc.tensor.transpose(
                        tp[:, 0:128],
                        Y[:, j, cc * 128 : (cc + 1) * 128],
                        ident[:],
                    )
                    nc.vector.tensor_copy(
                        yT[:, cc, j * 128 : (j + 1) * 128], tp[:, 0:128]
                    )
                    nc.vector.bn_stats(
                        out=stats[:, j, cc, :],
                        in_=Y[:, j, cc * 128 : (cc + 1) * 128],
                    )

                def emit_chunk_finish(cc):
                    for j in range(QT):
                        emit_chunk_finish_j(cc, j)

                # ---- one-shot work units + paced global queue ----
                # Every prefetched GEMM is a one-shot closure. The pair
                # that OWNS a unit force-runs it at the latest safe point
                # (deadline hooks inside its own chains); paced pops from
                # the global queue run units early when PE has slack, so
                # leftover work naturally flows into the last pairs where
                # the exp stream would otherwise throttle PE.
                def unit(fn):
                    st = [False]

                    def run():
                        if st[0]:
                            return False
                        st[0] = True
                        fn()
                        return True

                    run.consume = lambda: st.__setitem__(0, True)
                    return run

                global_q = deque()
                defer_q = deque()

                def pop1(_=None):
                    while global_q:
                        if global_q.popleft()():
                            return
                    while defer_q:
                        if defer_q.popleft()():
                            return

                def drain_all():
                    while global_q:
                        global_q.popleft()()
                    while defer_q:
                        defer_q.popleft()()

                PACE = int(os.environ.get("PACE", "1"))

                class PU:
                    pass

                def make_pair_units(wq1s, wk1s, wq2s8, wk2s8, wv1s,
                                    q1t, k1t, v1t, v8t,
                                    q2f8t, k2f8t, q28t, k28t, par,
                                    piecewise=False):
                    pu = PU()
                    pu.q1 = unit(lambda: emit_q_fill(q1t, wq1s))
                    pu.k1 = [
                        unit(lambda m=m: emit_k_gemm(k1t, wk1s, m))
                        for m in range(4)
                    ]
                    pu.v1 = [
                        unit(lambda u=u: emit_v1_unit(v1t, v8t, wv1s, u))
                        for u in range(8)
                    ]

                    def q2fn():
                        emit_q_fill8(q2f8t, wq2s8, par)
                        if piecewise:
                            emit_q2_fold(q28t, par)

                    def k2fn(m):
                        def f():
                            emit_k_gemm8(k2f8t, wk2s8, m, par)
                            if piecewise:
                                # same-queue FIFO: out(m) precedes in(m)
                                emit_k2_fold(k28t, par,
                                             m * 512, (m + 1) * 512)

                        return f

                    pu.q2 = unit(q2fn)
                    pu.k2 = [unit(k2fn(m)) for m in range(4)]
                    if piecewise:
                        pu.k2f = unit(lambda: None)
                        pu.q2f = unit(lambda: None)
                    else:
                        pu.k2f = unit(lambda: emit_k2_fold(k28t, par))
                        pu.q2f = unit(lambda: emit_q2_fold(q28t, par))
                    return pu

                def c1_qk_hook(pu, jit=False):
                    def h(g):
                        pu.q1()
                        for m in range(g // 2 + 1):
                            pu.k1[m]()
                        if jit:
                            # pair 0: wv1/x8/w28 DMAs land after xTb; forcing
                            # v1/k2 here would stall the in-order PE on DMA
                            # waits while QK work is ready. v1 is forced
                            # across the 4th QK phase; k2/q2/folds at c2 g0.
                            return
                        if g >= 2:
                            pu.v1[g - 2]()
                        # fp8 k2/q2 pipeline: GEMMs + bounce-out by g==7,
                        # fold-in DMAs issued right after (chain2 needs the
                        # folded tiles before its first QK matmul)
                        if g == 1:
                            pu.q2()
                        if g % 2 == 1:
                            pu.k2[(g - 1) // 2]()
                        if g == 7:
                            pu.k2f()
                            pu.q2f()
                        if qk_pop_ok(g):
                            pop1()

                    return h

                def c1_av_hook(pu):
                    def h(j):
                        if j == 0:
                            for u in pu.v1:
                                u()
                        else:
                            pop1()

                    return h

                def c2_qk_hook(pu):
                    def h(g):
                        if g == 0:
                            # safety: idempotent if already forced in chain1
                            pu.q2()
                            for u in pu.k2:
                                u()
                            pu.k2f()
                            pu.q2f()
                        if g % PACE == 1 and not (
                            qk_nodefer[0] and not global_q
                        ):
                            pop1()

                    return h

                QKPOP = os.environ.get("QKPOP", "none")

                def qk_pop_ok(g):
                    if QKPOP == "none":
                        return False
                    if QKPOP == "g7":
                        return g == 7
                    if QKPOP == "odd":
                        return g % 2 == 1
                    return True

                DEFPOP = int(os.environ.get("DEFPOP", "1"))
                # guard: during the LAST pair's QK phases the global queue
                # is empty and every pop would stuff a ~1us deferred unit
                # (proj stage1 etc.) between QK groups, stalling the
                # ACT-critical exp stream; hold them for the AV phases
                qk_nodefer = [False]

                def paced_qk(g):
                    if qk_pop_ok(g):
                        pop1()
                    elif DEFPOP and defer_q and not global_q \
                            and not qk_nodefer[0]:
                        pop1()

                AVJ = int(os.environ.get("AVJ", "2"))

                def paced_av(j):
                    if j >= AVJ:
                        pop1()

                # ---- attention chain, split into QK and AV phases so
                # chain2's QK (and its exp stream) overlaps chain1's AV ----
                def qk_phase(hh, kt, qp, qk_hook=None,
                             fp8av=False, k8q8=None):
                    po = hh * HD
                    ets = []
                    for g in range(8):
                        if qk_hook is not None:
                            qk_hook(g)
                        qk = psp.tile(
                            [128, 2, 512], f32, tag="qk", bufs=2, name="qk"
                        )
                        for g2 in range(2):
                            mc = g * 2 + g2
                            if k8q8 is not None:
                                k28t, q28t = k8q8
                                nc.tensor.matmul(
                                    qk[:, g2, :],
                                    k28t[:, hh, :, mc * 128 : (mc + 1) * 128],
                                    q28t[:, hh],
                                    start=True,
                                    stop=True,
                                    perf_mode=DR,
                                )
                            else:
                                nc.tensor.matmul(
                                    qk[:, g2, :],
                                    kt[po : po + 64, mc * 128 : (mc + 1) * 128],
                                    qp[po : po + 64, :],
                                    start=True,
                                    stop=True,
                                )
                        if fp8av:
                            e_t = ep.tile(
                                [128, 2, 512], fp8, tag="E8", name="e8_t"
                            )
                            nc.scalar.activation(
                                e_t[:],
                                qk[:],
                                mybir.ActivationFunctionType.Exp,
                                bias=esh_t[:],
                                scale=0.125,
                            )
                        else:
                            e_t = ep.tile(
                                [128, 2, 512], bf16, tag="E", name="e_t"
                            )
                            nc.scalar.activation(
                                e_t[:],
                                qk[:],
                                mybir.ActivationFunctionType.Exp,
                                scale=0.125,
                            )
                        ets.append(e_t)
                    return ets

                def av_phase(hh, v1p, ets, av_hook=None, fp8av=False):
                    av = psp.tile(
                        [128, QT, HD + 1], f32, tag="av", bufs=2, name="av"
                    )
                    for j in range(QT):
                        if av_hook is not None:
                            av_hook(j)
                        if fp8av:
                            for g in range(8):
                                nc.tensor.matmul(
                                    av[:, j, :],
                                    ets[g][:, :, j * 128 : (j + 1) * 128],
                                    v1p[:, 2 * g : 2 * g + 2, hh, :],
                                    start=(g == 0),
                                    stop=(g == 7),
                                    perf_mode=DR,
                                )
                        else:
                            for mc in range(16):
                                nc.tensor.matmul(
                                    av[:, j, :],
                                    ets[mc // 2][:, mc % 2, j * 128 : (j + 1) * 128],
                                    v1p[:, mc, hh, :],
                                    start=(mc == 0),
                                    stop=(mc == 15),
                                )
                    return av

                def combine(h, av1, av2):
                    # reads the AV psums directly; av1 released first so
                    # the next chain can reuse the psum buffer sooner
                    r1 = wpool.tile([128, QT, 1], f32, tag="r1", bufs=2, name="r1")
                    nc.vector.reciprocal(r1[:], av1[:, :, HD : HD + 1])
                    t1 = wpool.tile([128, QT, HD], f32, tag="t1", bufs=1, name="t1")
                    r1b = bass.AP(
                        tensor=r1.tensor,
                        offset=r1.offset,
                        ap=[r1.ap[0], r1.ap[1], [0, HD]],
                    )
                    nc.vector.tensor_tensor(
                        out=t1[:],
                        in0=av1[:, :, 0:HD],
                        in1=r1b,
                        op=mybir.AluOpType.mult,
                    )
                    r2 = wpool.tile([128, QT, 1], f32, tag="r2", bufs=2, name="r2")
                    nc.vector.reciprocal(r2[:], av2[:, :, HD : HD + 1])
                    lam_h = lam_b[:, h : h + 1]
                    lam_bc = bass.AP(
                        tensor=lam_h.tensor,
                        offset=lam_h.offset,
                        ap=[lam_h.ap[0], [0, QT], [0, 1]],
                    )
                    nc.vector.tensor_tensor(
                        out=r2[:], in0=r2[:], in1=lam_bc, op=mybir.AluOpType.mult
                    )
                    t2 = wpool.tile([128, QT, HD], f32, tag="t2", bufs=1, name="t2")
                    r2b = bass.AP(
                        tensor=r2.tensor,
                        offset=r2.offset,
                        ap=[r2.ap[0], r2.ap[1], [0, HD]],
                    )
                    nc.vector.tensor_tensor(
                        out=t2[:],
                        in0=av2[:, :, 0:HD],
                        in1=r2b,
                        op=mybir.AluOpType.mult,
                    )
                    nc.vector.tensor_tensor(
                        out=Y[:, :, h * 64 : (h + 1) * 64],
                        in0=t1[:],
                        in1=t2[:],
                        op=mybir.AluOpType.subtract,
                    )

                # ---- startup DMAs (issue order = DMA device order) ----
                # chain1 QK of pair 0 paces the whole ramp: its xTb/w1
                # stream goes first; wv1 (needed at c1AV) and the fp8 side
                # (needed at c2QK, ~8 exp-groups later) follow
                dma_xTb(0, 256)
                wq1 = dma_wslice("wq1", w1_d, 0)
                dma_xTb(256, 512)
                wk1 = dma_wslice("wk1", w1_d, DIM)
                dma_xTb(512, 1024)
                dma_xTb(1024, 1536)
                dma_xTb(1536, 2048)
                dma_xT8(0, 1024)
                wq2 = dma_w8slice("wq2", 0)
                wk2 = dma_w8slice("wk2", DIM)
                dma_xT8(1024, 2048)
                wv1 = dma_wslice("wv1", wv1_d, 0, bf16)

                def alloc_pair_fp8():
                    q2f8t = wpool.tile([128, NQ], fp8, tag="q2f8", name="q2f8")
                    k2f8t = wpool.tile([128, N], fp8, tag="k2f8", name="k2f8")
                    q28t = wpool.tile([32, 2, 2, NQ], fp8, tag="q28", name="q28")
                    k28t = wpool.tile([32, 2, 2, N], fp8, tag="k28", name="k28")
                    return q2f8t, k2f8t, q28t, k28t

                # ---- pair-0 prologue ----
                q1p = alloc_q("q1p")
                emit_q_fill(q1p, wq1, 0, 256)
                emit_q_fill(q1p, wq1, 256, 512)
                k1T = alloc_k("k1T")
                v1p = alloc_v1()
                v18p = alloc_v18()
                q2f8, k2f8, q28, k28 = alloc_pair_fp8()
                pu = make_pair_units(wq1, wk1, wq2, wk2, wv1,
                                     q1p, k1T, v1p, v18p,
                                     q2f8, k2f8, q28, k28, par=0,
                                     piecewise=True)
                pu.q1.consume()  # filled above in halves
                pu.k1[0]()

                # ---- pair loop ----
                for p in range(NPAIR):
                    nx = p + 1
                    if nx < NPAIR:
                        wq1n = dma_wslice("wq1", w1_d, nx * 128)
                        wk1n = dma_wslice("wk1", w1_d, DIM + nx * 128)
                        wq2n = dma_w8slice("wq2", nx * 128)
                        wk2n = dma_w8slice("wk2", DIM + nx * 128)
                        wv1n = dma_wslice("wv1", wv1_d, nx * 128, bf16)
                        if nx == 4:
                            nc.sync.dma_start(
                                wpj[:],
                                wp_d[:, :].rearrange("(c p) n -> p c n", p=128),
                            )
                        q1pn = alloc_q("q1p")
                        k1Tn = alloc_k("k1T")
                        v1pn = alloc_v1()
                        v18pn = alloc_v18()
                        q2f8n, k2f8n, q28n, k28n = alloc_pair_fp8()
                        pun = make_pair_units(wq1n, wk1n, wq2n, wk2n, wv1n,
                                              q1pn, k1Tn, v1pn, v18pn,
                                              q2f8n, k2f8n, q28n, k28n,
                                              par=nx % 2)
                        global_q.append(pun.q1)
                        for u in pun.k1:
                            global_q.append(u)
                        for u in pun.v1:
                            global_q.append(u)
                        global_q.append(pun.q2)
                        for u in pun.k2:
                            global_q.append(u)
                        global_q.append(pun.k2f)
                        global_q.append(pun.q2f)
                        if p >= 1:
                            defer_q.append(
                                unit(lambda cc=p - 1: emit_chunk_finish(cc))
                            )
                    elif p == NPAIR - 1:
                        defer_q.append(unit(lambda: emit_chunk_finish(4)))
                        for j in range(QT):
                            for half in range(2):
                                defer_q.append(
                                    unit(
                                        lambda jj=j, hh=half: emit_proj_stage1(
                                            jj, hh
                                        )
                                    )
                                )

                    # all four QK phases first: 16 bf16 exps of run-ahead
                    # cover the fp8 side's GEMM+bounce+fold latency, and
                    # the exp stream never starves at phase boundaries
                    qk_nodefer[0] = p == NPAIR - 1

                    def p0_c2b_hook(g, pu=pu):
                        pu.v1[g]()  # wv1 has landed by the 4th QK phase
                        paced_qk(g)

                    ets1a = qk_phase(0, k1T, q1p,
                                     qk_hook=c1_qk_hook(pu, jit=(p == 0)))
                    ets1b = qk_phase(1, k1T, q1p, qk_hook=paced_qk)
                    ets2a = qk_phase(0, None, None, qk_hook=c2_qk_hook(pu),
                                     fp8av=True, k8q8=(k28, q28))
                    ets2b = qk_phase(1, None, None,
                                     qk_hook=(p0_c2b_hook if p == 0
                                              else paced_qk),
                                     fp8av=True, k8q8=(k28, q28))
                    qk_nodefer[0] = False
                    av1 = av_phase(0, v1p, ets1a, av_hook=c1_av_hook(pu))
                    av2 = av_phase(0, v18p, ets2a, av_hook=paced_av,
                                   fp8av=True)
                    combine(2 * p, av1, av2)
                    if p == NPAIR - 1:
                        defer_q.append(unit(lambda: emit_head_finish(10)))
                    av1 = av_phase(1, v1p, ets1b, av_hook=paced_av)
                    av2 = av_phase(1, v18p, ets2b, av_hook=paced_av,
                                   fp8av=True)
                    combine(2 * p + 1, av1, av2)

                    if nx < NPAIR:
                        k1T, v1p, v18p = k1Tn, v1pn, v18pn
                        q1p = q1pn
                        q28, k28 = q28n, k28n
                        pu = pun

                drain_all()

            # ---- tail: per query tile: chunk-5 finish -> rms -> proj ----
            with tc.tile_pool(name="proj", bufs=1) as prj:
                mv = prj.tile([128, QT, 2], f32, tag="mv")
                rms = prj.tile([128, QT], f32, tag="rms")
                eps_t = prj.tile([128, 1], f32, tag="eps_t")
                nc.vector.memset(eps_t[:], EPS)

                def emit_rms_j(j):
                    nc.vector.bn_aggr(out=mv[:, j, :], in_=stats[:, j])

                    nc.vector.tensor_tensor(
                        out=mv[:, j, 0:1],
                        in0=mv[:, j, 0:1],
                        in1=mv[:, j, 0:1],
                        op=mybir.AluOpType.mult,
                    )
                    nc.vector.tensor_tensor(
                        out=mv[:, j, 1:2],
                        in0=mv[:, j, 1:2],
                        in1=mv[:, j, 0:1],
                        op=mybir.AluOpType.add,
                    )
                    nc.scalar.activation(
                        rms[:, j : j + 1],
                        mv[:, j, 1:2],
                        mybir.ActivationFunctionType.Sqrt,
                        bias=eps_t[:],
                        scale=1.0,
                    )
                    nc.vector.reciprocal(rms[:, j : j + 1], rms[:, j : j + 1])

                def emit_proj_j(j):
                    # stage 2: chunk-5 contribution joins the stage-1
                    # accumulator; rms folds into the epilogue:
                    # (y*rms) @ Wp = (y @ Wp) * rms  (rms is per-token)
                    jr = slice(j * 128, (j + 1) * 128)
                    osb2 = prj.tile(
                        [128, DIM], f32, tag="out_sb", bufs=2, name="osb2"
                    )
                    osb3 = prj.tile(
                        [128, DIM], f32, tag="out_sb3", bufs=2, name="osb3"
                    )
                    for half in range(2):
                        ps = psp.tile([128, 512], f32, tag="mm", bufs=2, name="psp2")
                        nc.tensor.matmul(
                            ps[:, 0:384],
                            yT[:, 5, jr],
                            wpj[:, 5, half * 384 : (half + 1) * 384],
                            start=True,
                            stop=True,
                        )
                        hs = slice(half * 384, (half + 1) * 384)
                        nc.vector.tensor_tensor(
                            out=osb2[:, hs],
                            in0=pacc[:, j, hs],
                            in1=ps[:, 0:384],
                            op=mybir.AluOpType.add,
                        )
                        # rms scale on ACT: the exp stream is finished by
                        # the tail, so ACT is free while DVE is the serial
                        # bottleneck of the rms/epilogue chain
                        nc.scalar.activation(
                            osb3[:, hs],
                            osb2[:, hs],
                            mybir.ActivationFunctionType.Copy,
                            scale=rms[:, j : j + 1],
                        )
                        nc.sync.dma_start(
                            out_d[j * 128 : (j + 1) * 128, hs], osb3[:, hs]
                        )

                for j in range(QT):
                    emit_head_finish_j(11, j, tail=True)
                    emit_rms_j(j)
                    emit_proj_j(j)

    _split_waits(nc)
    return nc


def kernel(x, W_qkv1, W_qkv2, W_proj, b_proj, norm_w, lambda_1, lambda_2, xpos):
    import ml_dtypes
    from concourse.bass_utils import run_bass_kernel_spmd

    if "nc" not in _cache:
        _cache["nc"] = _build()
    nc = _cache["nc"]

    bf16 = ml_dtypes.bfloat16
    e4m3 = ml_dtypes.float8_e4m3
    x = np.asarray(x, dtype=np.float32)
    w1 = np.asarray(W_qkv1, dtype=np.float32)
    w2 = np.asarray(W_qkv2, dtype=np.float32)
    w1qk = np.ascontiguousarray(w1[:, : 2 * DIM]).astype(bf16)
    w28 = np.clip(w2[:, : 2 * DIM], -240, 240).astype(e4m3)
    wv1b = np.ascontiguousarray(w1[:, 2 * DIM :]).astype(bf16)
    wpb = np.ascontiguousarray(
        np.asarray(norm_w, dtype=np.float32)[:, None]
        * np.asarray(W_proj, dtype=np.float32)
    ).astype(bf16)
    bp = np.ascontiguousarray(np.asarray(b_proj, dtype=np.float32))
    lam = np.ascontiguousarray(
        (
            np.asarray(lambda_1, dtype=np.float32)
            - np.asarray(lambda_2, dtype=np.float32)
            + LAMBDA_INIT
        ).astype(np.float32)
    )

    in_maps = []
    for c in range(NCORES):
        b, qi = c // 4, c % 4
        # rotate tokens so this core's query block comes first; attention
        # is permutation-invariant over keys so only q/out order matters
        xr = np.roll(x[b], -qi * NQ, axis=0)
        xrT = np.ascontiguousarray(xr.T)
        in_maps.append(
            {
                "xb": xrT.astype(bf16),
                "x8": np.clip(xrT, -240, 240).astype(e4m3),
                "w1": w1qk,
                "w28": w28,
                "wv1b": wv1b,
                "wp": wpb,
                "bp": bp,
                "lam": lam,
            }
        )

    res = run_bass_kernel_spmd(nc, in_maps, core_ids=list(range(NCORES)))
    out = np.empty((B, N, DIM), dtype=np.float32)
    for c in range(NCORES):
        b, qi = c // 4, c % 4
        out[b, qi * NQ : (qi + 1) * NQ, :] = res.results[c]["out"]
    return out

